# revision 1
# baseline (speedup 1.0000x reference)
"""Distributed GCN (5x GraphConv(add) + residual/ReLU + mean-pool + linear)
for 8 Trainium2 NeuronCores.

Sharding: nodes partitioned contiguously across cores (1280 nodes/core, padded
to 10240). Each core owns the edges whose *destination* lands in its shard.
Aggregation is computed as A@(x@Wr): project first (p = x@Wr), AllGather p,
gather p[src[e]] rows with SWDGE dma_gather, then reduce edge tiles onto
destination nodes with one-hot segment matmuls on the tensor engine.
x@Ws + bias accumulate into the same PSUM bank; residual+ReLU on DVE/ACT.
Mean-pool partials via matmul against a node->graph one-hot, AllReduce,
final linear on-chip. Everything in bf16 with fp32 PSUM accumulation.
"""

import numpy as np
import ml_dtypes

BF16 = ml_dtypes.bfloat16

N, E, D, OUT, G = 10000, 160000, 512, 128, 64
NCORES, P = 8, 128
NBLK = 10                     # 128-node blocks per core
NC_NODES = NBLK * P           # 1280
NPAD = NCORES * NC_NODES      # 10240
NLAYERS = 5
KD = D // P                   # 4 chunks of in-channels


def _wrap_idx(a):
    """[L] ints -> [128, L//16] int16 SWDGE index layout (16-partition wrap,
    replicated for the 8 Q7 cores)."""
    L = len(a)
    w = a.astype(np.int16).reshape(L // 16, 16).T
    return np.ascontiguousarray(np.tile(w, (8, 1)))


def _prep(inputs):
    x = np.asarray(inputs["x"], np.float32)
    ei = np.asarray(inputs["edge_index"]).astype(np.int64)
    batch = np.asarray(inputs["batch"]).astype(np.int64)
    src, dst = ei[0], ei[1]

    order = np.argsort(dst, kind="stable")
    ds_, ss_ = dst[order], src[order]
    starts = np.searchsorted(ds_, np.arange(0, NPAD + 1, P))
    counts = np.diff(starts)
    T_pad = max(1, int(np.ceil(counts.max() / P)))
    L = T_pad * P

    xp = np.zeros((NPAD, D), np.float32)
    xp[:N] = x

    counts_g = np.bincount(batch, minlength=G)[:G]
    inv = (1.0 / np.maximum(counts_g, 1.0)).astype(np.float32)

    per_core = []
    for c in range(NCORES):
        idx_blocks = []
        oh_flat = np.zeros((P, NBLK * T_pad * P), BF16)
        goh = np.zeros((P, NBLK * G), BF16)
        for b in range(NBLK):
            gb = c * NBLK + b
            lo = gb * P
            s0, s1 = int(starts[gb]), int(starts[gb + 1])
            n = s1 - s0
            srcs = np.zeros(L, np.int64)
            srcs[:n] = ss_[s0:s1]
            dloc = ds_[s0:s1] - lo
            oh = np.zeros((L, P), BF16)
            oh[np.arange(n), dloc] = 1
            idx_blocks.append(_wrap_idx(srcs))
            oh_flat[:, b * T_pad * P:(b + 1) * T_pad * P] = (
                oh.reshape(T_pad, P, P).transpose(1, 0, 2).reshape(P, T_pad * P))
            nodes = lo + np.arange(P)
            valid = nodes < N
            goh[valid, b * G + batch[nodes[valid]]] = 1

        shard = xp[c * NC_NODES:(c + 1) * NC_NODES].astype(BF16)
        xt0 = np.ascontiguousarray(
            shard.T.reshape(KD, P, NC_NODES).transpose(1, 0, 2))
        per_core.append(dict(
            x_shard=np.ascontiguousarray(shard),
            xt0=xt0,
            ohot=oh_flat,
            idxe=np.ascontiguousarray(np.concatenate(idx_blocks, axis=1)),
            goh=goh,
        ))

    wr = np.zeros((P, NLAYERS, KD, D), BF16)
    ws = np.zeros((P, NLAYERS, KD, D), BF16)
    bias = np.zeros((P, NLAYERS, D), BF16)
    for l in range(NLAYERS):
        wr[:, l] = np.asarray(inputs[f"Wr{l+1}"], np.float32).reshape(
            KD, P, D).transpose(1, 0, 2).astype(BF16)
        ws[:, l] = np.asarray(inputs[f"Ws{l+1}"], np.float32).reshape(
            KD, P, D).transpose(1, 0, 2).astype(BF16)
        bias[0, l] = np.asarray(inputs[f"b{l+1}"], np.float32).astype(BF16)
    ones_e0 = np.zeros((P, P), BF16)
    ones_e0[0, :] = 1
    wlin = np.ascontiguousarray(
        np.asarray(inputs["Wlin"], np.float32).reshape(KD, P, OUT)
        .transpose(1, 0, 2).astype(BF16))
    blin = np.tile(np.asarray(inputs["blin"], np.float32).reshape(OUT, 1),
                   (1, 1)).astype(np.float32)
    shared = dict(
        wr=wr, ws=ws, bias=bias, ones=ones_e0, wlin=wlin, blin=blin,
        invt=np.ascontiguousarray(np.tile(inv, (P, KD)).astype(np.float32)),
        ident=np.eye(P, dtype=BF16),
    )
    return per_core, shared, T_pad


def _unwrap(w, L):
    """inverse of _wrap_idx: [128, L//16] -> [L]"""
    return np.ascontiguousarray(w[:16].T).reshape(-1)[:L].astype(np.int64)


def emulate(inputs):
    """Numpy emulation of the exact device dataflow (bf16 casts included).
    Validates all host-side index/one-hot bookkeeping."""
    per_core, shared, T_pad = _prep(inputs)
    L = T_pad * P
    f32 = np.float32

    xs = [pc["x_shard"].astype(f32) for pc in per_core]       # [1280, 512]
    for l in range(NLAYERS):
        wr_l = np.concatenate([shared["wr"][:, l, k, :] for k in range(KD)],
                              axis=0).astype(f32)             # [512, 512]
        ws_l = np.concatenate([shared["ws"][:, l, k, :] for k in range(KD)],
                              axis=0).astype(f32)
        b_l = shared["bias"][0, l].astype(f32)
        # p = x @ Wr, cast bf16, "AllGather"
        p_full = np.concatenate(
            [(xs[c] @ wr_l).astype(BF16).astype(f32) for c in range(NCORES)],
            axis=0)                                           # [10240, 512]
        new_xs = []
        for c in range(NCORES):
            nx = np.zeros((NC_NODES, D), f32)
            for b in range(NBLK):
                idx = _unwrap(
                    per_core[c]["idxe"][:, b * (L // 16):(b + 1) * (L // 16)], L)
                gath = p_full[idx].astype(BF16).astype(f32)   # [L, 512]
                acc = np.zeros((P, D), f32)
                for t in range(T_pad):
                    oh = per_core[c]["ohot"][
                        :, (b * T_pad + t) * P:(b * T_pad + t + 1) * P
                    ].astype(f32)                             # [128e, 128d]
                    acc += oh.T @ gath[t * P:(t + 1) * P]
                blk = xs[c][b * P:(b + 1) * P]
                acc += blk @ ws_l + b_l
                val = (acc.astype(f32) + blk)
                if l < NLAYERS - 1:
                    val = np.maximum(val, 0)
                nx[b * P:(b + 1) * P] = val.astype(BF16).astype(f32)
            new_xs.append(nx)
        xs = new_xs
    # pooling
    pooled_T = np.zeros((D, G), f32)
    for c in range(NCORES):
        goh = per_core[c]["goh"].astype(f32)
        for b in range(NBLK):
            blk = xs[c][b * P:(b + 1) * P].astype(BF16).astype(f32)
            for j in range(KD):
                pooled_T[j * P:(j + 1) * P] += (
                    blk[:, j * P:(j + 1) * P].T @ goh[:, b * G:(b + 1) * G])
    inv = shared["invt"][0, :G].astype(f32)
    pooled_T = (pooled_T * inv[None, :]).astype(BF16).astype(f32)
    wlin = np.concatenate([shared["wlin"][:, k, :] for k in range(KD)],
                          axis=0).astype(f32)                 # [512, 128]
    out_T = wlin.T @ pooled_T + shared["blin"][:, :1]         # [128, 64]
    return np.ascontiguousarray(out_T.T).astype(np.float32)


def _build(T_pad, enable_asserts=False):
    import os
    n_layers = int(os.environ.get("GCN_LAYERS", NLAYERS))
    no_gather = bool(int(os.environ.get("GCN_NO_GATHER", "0")))
    no_cc = bool(int(os.environ.get("GCN_NO_CC", "0")))
    bP, bA, bT = (int(v) for v in os.environ.get("GCN_BANKS", "1,1,2").split(","))
    gbufs = int(os.environ.get("GCN_GBUFS", "3"))
    gsplit = int(os.environ.get("GCN_GSPLIT", "5"))
    seg_stride = int(os.environ.get("GCN_SEG_STRIDE", "1"))  # timing expts only
    no_tr = bool(int(os.environ.get("GCN_NO_TR", "0")))      # timing expts only
    import concourse.bass as bass
    import concourse.mybir as mybir
    import concourse.tile as tile
    from concourse import bacc

    F32 = mybir.dt.float32
    BF = mybir.dt.bfloat16
    I16 = mybir.dt.int16
    ADD = mybir.AluOpType.add
    MUL = mybir.AluOpType.mult
    L = T_pad * P
    RG = [list(range(NCORES))]

    nc = bacc.Bacc("TRN2", target_bir_lowering=False, debug=False,
                   enable_asserts=enable_asserts, num_devices=NCORES)

    # per-core inputs
    x_d = nc.dram_tensor("x_shard", [NC_NODES, D], BF, kind="ExternalInput")
    xt0_d = nc.dram_tensor("xt0", [P, KD, NC_NODES], BF, kind="ExternalInput")
    oh_d = nc.dram_tensor("ohot", [P, NBLK * T_pad * P], BF, kind="ExternalInput")
    idxe_d = nc.dram_tensor("idxe", [P, NBLK * (L // 16)], I16, kind="ExternalInput")
    goh_d = nc.dram_tensor("goh", [P, NBLK * G], BF, kind="ExternalInput")
    # shared inputs
    wr_d = nc.dram_tensor("wr", [P, NLAYERS, KD, D], BF, kind="ExternalInput")
    ws_d = nc.dram_tensor("ws", [P, NLAYERS, KD, D], BF, kind="ExternalInput")
    bias_d = nc.dram_tensor("bias", [P, NLAYERS, D], BF, kind="ExternalInput")
    ones_d = nc.dram_tensor("ones", [P, P], BF, kind="ExternalInput")
    wlin_d = nc.dram_tensor("wlin", [P, KD, OUT], BF, kind="ExternalInput")
    blin_d = nc.dram_tensor("blin", [OUT, 1], F32, kind="ExternalInput")
    invt_d = nc.dram_tensor("invt", [P, KD * G], F32, kind="ExternalInput")
    ident_d = nc.dram_tensor("ident", [P, P], BF, kind="ExternalInput")
    # internal DRAM (double-buffered by layer parity so the AllGather for
    # layer l+1 never WAR-depends on layer l's gathers)
    p_shard = [nc.dram_tensor(f"p_shard{i}", [NC_NODES, D], BF) for i in (0, 1)]
    p_full = [nc.dram_tensor(f"p_full{i}", [NPAD, D], BF, addr_space="Shared")
              for i in (0, 1)]
    pool_in = nc.dram_tensor("pool_in", [P, KD * G], F32)
    pool_out = nc.dram_tensor("pool_out", [P, KD * G], F32, addr_space="Shared")
    # output
    out_d = nc.dram_tensor("out_t", [OUT, G], F32, kind="ExternalOutput")

    with tile.TileContext(nc) as tc:
        with (
            tc.tile_pool(name="const", bufs=1) as const,
            tc.tile_pool(name="xs", bufs=2) as xpool,
            tc.tile_pool(name="xt", bufs=2) as xtpool,
            tc.tile_pool(name="gath", bufs=gbufs) as gpool,
            tc.tile_pool(name="small", bufs=int(os.environ.get("GCN_SBUFS", "4"))) as spool,
            tc.tile_pool(name="psP", bufs=bP, space="PSUM") as psP,
            tc.tile_pool(name="psA", bufs=bA, space="PSUM") as psA,
            tc.tile_pool(name="psS", bufs=1, space="PSUM") as psS,
            tc.tile_pool(name="psT", bufs=bT, space="PSUM") as psT,
        ):
            # ---- constants to SBUF
            oh_sb = const.tile([P, NBLK * T_pad * P], BF, tag="oh")
            nc.sync.dma_start(oh_sb[:], oh_d[:])
            idxe_sb = const.tile([P, NBLK * (L // 16)], I16, tag="idxe")
            nc.sync.dma_start(idxe_sb[:], idxe_d[:])
            ident_sb = const.tile([P, P], BF, tag="ident")
            nc.sync.dma_start(ident_sb[:], ident_d[:])
            goh_sb = const.tile([P, NBLK * G], BF, tag="goh")
            nc.sync.dma_start(goh_sb[:], goh_d[:])
            wr_sb = const.tile([P, NLAYERS, KD, D], BF, tag="wr")
            nc.sync.dma_start(wr_sb[:], wr_d[:])
            ws_sb = const.tile([P, NLAYERS, KD, D], BF, tag="ws")
            nc.sync.dma_start(ws_sb[:], ws_d[:])
            bias_sb = const.tile([P, NLAYERS, D], BF, tag="bias")
            nc.sync.dma_start(bias_sb[:], bias_d[:])
            ones_sb = const.tile([P, P], BF, tag="ones")
            nc.sync.dma_start(ones_sb[:], ones_d[:])
            wlin_sb = const.tile([P, KD, OUT], BF, tag="wlin")
            nc.sync.dma_start(wlin_sb[:], wlin_d[:])
            blin_sb = const.tile([OUT, 1], F32, tag="blin")
            nc.sync.dma_start(blin_sb[:], blin_d[:])
            invt_sb = const.tile([P, KD * G], F32, tag="invt")
            nc.sync.dma_start(invt_sb[:], invt_d[:])

            xs_cur = xpool.tile([P, NBLK, D], BF, tag="xs")
            nc.sync.dma_start(xs_cur[:], x_d.ap().rearrange("(b p) d -> p b d", p=P))
            xt_cur = xtpool.tile([P, KD, NC_NODES], BF, tag="xt")
            nc.sync.dma_start(xt_cur[:], xt0_d[:])

            def emit_p_block(xt_src, layer, m, pbuf):
                """p[l=layer] block m = x_l[block m] @ Wr_l, into p_shard[pbuf]."""
                pps = psP.tile([P, D], F32, tag="pps", name=f"pps_{layer}_{m}")
                for k in range(KD):
                    nc.tensor.matmul(
                        pps[:],
                        lhsT=xt_src[:, k, m * P:(m + 1) * P],
                        rhs=wr_sb[:, layer, k, :],
                        start=(k == 0), stop=(k == KD - 1))
                p_sb = spool.tile([P, D], BF, tag="psb", name=f"psb_{layer}_{m}")
                nc.vector.tensor_copy(p_sb[:], pps[:])
                nc.sync.dma_start(
                    p_shard[pbuf][m * P:(m + 1) * P, :], p_sb[:])

            def emit_ag(pbuf):
                if no_cc:
                    nc.sync.dma_start(
                        p_full[pbuf][:NC_NODES, :], p_shard[pbuf][:])
                else:
                    nc.gpsimd.collective_compute(
                        "AllGather", mybir.AluOpType.bypass, replica_groups=RG,
                        ins=[p_shard[pbuf][:]], outs=[p_full[pbuf][:]])

            # prologue: projection for layer 0
            for m in range(NBLK):
                emit_p_block(xt_cur, 0, m, 0)
            emit_ag(0)

            pool_ps = [
                psS.tile([P, G], F32, tag=f"pool{j}", name=f"pool_ps{j}")
                for j in range(KD)
            ]
            for l in range(n_layers):
                pbuf = l % 2
                xs_next = xpool.tile([P, NBLK, D], BF, tag="xs")
                last = l == NLAYERS - 1
                if not last:
                    xt_next = xtpool.tile([P, KD, NC_NODES], BF, tag="xt")
                for b in range(NBLK):
                    g = gpool.tile([P, T_pad, D], BF, tag="g")
                    if no_gather:
                        nc.vector.memset(g[:], 0)
                    else:
                        # split the block gather so segment matmuls on early
                        # tiles overlap later chunks' DMA drain
                        nsp = min(gsplit, T_pad)
                        th = (T_pad + nsp - 1) // nsp
                        col0 = b * (L // 16)
                        for s0 in range(0, T_pad, th):
                            s1 = min(s0 + th, T_pad)
                            nc.gpsimd.dma_gather(
                                g[:, s0:s1, :], p_full[pbuf][:],
                                idxe_sb[:, col0 + s0 * 8:col0 + s1 * 8],
                                (s1 - s0) * P, (s1 - s0) * P, D,
                                single_packet=False)
                    aps = psA.tile([P, D], F32, tag="aps")
                    # Ws + bias first: they only need resident data, so PE
                    # progresses on this block while its gather chunks drain
                    for k in range(KD):
                        nc.tensor.matmul(
                            aps[:],
                            lhsT=xt_cur[:, k, b * P:(b + 1) * P],
                            rhs=ws_sb[:, l, k, :],
                            start=(k == 0), stop=False)
                    nc.tensor.matmul(
                        aps[:], lhsT=ones_sb[:], rhs=bias_sb[:, l, :],
                        start=False, stop=False)
                    for ti, t in enumerate(range(0, T_pad, seg_stride)):
                        nc.tensor.matmul(
                            aps[:],
                            lhsT=oh_sb[:, (b * T_pad + t) * P:(b * T_pad + t + 1) * P],
                            rhs=g[:, t, :],
                            start=False,
                            stop=(t + seg_stride >= T_pad))
                    if last:
                        nc.vector.tensor_tensor(
                            xs_next[:, b, :], aps[:], xs_cur[:, b, :], op=ADD)
                        # pooling partials for this block, interleaved so they
                        # hide under later blocks' gathers
                        for j in range(KD):
                            nc.tensor.matmul(
                                pool_ps[j][:],
                                lhsT=xs_next[:, b, j * P:(j + 1) * P],
                                rhs=goh_sb[:, b * G:(b + 1) * G],
                                start=(b == 0), stop=(b == NBLK - 1))
                    else:
                        t1 = spool.tile([P, D], BF, tag="t1")
                        nc.vector.tensor_tensor(
                            t1[:], aps[:], xs_cur[:, b, :], op=ADD)
                        nc.scalar.activation(
                            xs_next[:, b, :], t1[:],
                            func=mybir.ActivationFunctionType.Relu)
                        # transpose new block into xt_next (channel-major)
                        if no_tr:
                            nc.vector.tensor_copy(
                                xt_next[:, :, b * P:(b + 1) * P],
                                xs_next[:, b, :].rearrange(
                                    "p (j q) -> p j q", j=KD)[:, :, :P])
                        else:
                            for j in range(KD):
                                trps = psT.tile([P, P], BF, tag="tr")
                                nc.tensor.transpose(
                                    trps[:], xs_next[:, b, j * P:(j + 1) * P],
                                    ident_sb[:])
                                nc.vector.tensor_copy(
                                    xt_next[:, j, b * P:(b + 1) * P], trps[:])
                        # pipelined projection for layer l+1, block b
                        emit_p_block(xt_next, l + 1, b, 1 - pbuf)
                if not last:
                    emit_ag(1 - pbuf)
                    xt_cur = xt_next
                xs_cur = xs_next

            # ---- pooling partials were accumulated inside the last layer's
            # block loop (one PSUM bank per 128-channel chunk)
            pool_sb = spool.tile([P, KD * G], F32, tag="pool_sb")
            for j in range(KD):
                nc.vector.tensor_copy(pool_sb[:, j * G:(j + 1) * G], pool_ps[j][:])
            nc.sync.dma_start(pool_in[:], pool_sb[:])
            if no_cc:
                nc.sync.dma_start(pool_out[:], pool_sb[:])
            else:
                nc.gpsimd.collective_compute(
                    "AllReduce", ADD, replica_groups=RG,
                    ins=[pool_in[:]], outs=[pool_out[:]])
            pool2 = spool.tile([P, KD * G], F32, tag="pool2")
            nc.sync.dma_start(pool2[:], pool_out[:])
            poolbf = spool.tile([P, KD * G], BF, tag="poolbf")
            nc.vector.tensor_tensor(poolbf[:], pool2[:], invt_sb[:], op=MUL)
            fin_ps = psS.tile([P, G], F32, tag="pool0", name="fin_ps")
            for k in range(KD):
                nc.tensor.matmul(
                    fin_ps[:], lhsT=wlin_sb[:, k, :],
                    rhs=poolbf[:, k * G:(k + 1) * G],
                    start=(k == 0), stop=(k == KD - 1))
            fin_sb = spool.tile([OUT, G], F32, tag="fin_sb")
            nc.vector.tensor_tensor(
                fin_sb[:], fin_ps[:], blin_sb[:, :1].to_broadcast([OUT, G]),
                op=ADD)
            nc.sync.dma_start(out_d[:], fin_sb[:])

    nc.compile()
    return nc


def kernel(**inputs):
    import os
    from concourse.bass_utils import run_bass_kernel_spmd

    per_core, shared, T_pad = _prep(inputs)
    nc = _build(T_pad)
    in_maps = [{**pc, **shared} for pc in per_core]
    trace = bool(int(os.environ.get("GCN_TRACE", "0")))
    res = run_bass_kernel_spmd(nc, in_maps, core_ids=list(range(NCORES)),
                               trace=trace)
    if trace:
        print(f"HW exec time: {res.exec_time_ns} ns")
        if res.instructions_and_trace is not None:
            print("trace:", res.instructions_and_trace[1])
    out_t = res.results[0]["out_t"]
    return np.ascontiguousarray(out_t.T).astype(np.float32)



# revision 32
# speedup vs baseline: 1.5856x; 1.5856x over previous
"""Distributed GCN (5x GraphConv(add) + residual/ReLU + mean-pool + linear)
for 8 Trainium2 NeuronCores.

Sharding: nodes are permuted so every 128-node block has a near-equal number
of incident (destination) edges, then partitioned contiguously across cores
(1280 nodes/core). Each core owns the edges whose destination lands in its
shard. Aggregation is computed as A@(x@Wr): project first (p = x@Wr),
AllGather p, gather p[src[e]] rows with SWDGE dma_gather, then reduce edge
tiles onto destination nodes with one-hot segment matmuls on the tensor
engine.

The aggregation path runs in fp8 e4m3: p is stored/AllGathered/gathered as
fp8 (|p| < 240 for this model scale) and the one-hot segment matmuls use
DoubleRow perf mode (K=256 contraction, 0.5 PE cycles/row = 2x bf16
throughput). Edge balancing makes T (128-edge tiles per block) uniform and
even, which DoubleRow pairing requires. x@Ws + bias accumulate into the same
PSUM bank; residual+ReLU on DVE/ACT. Weights and node features stay bf16.

The AllGather is split 1024+256 rows per core: the big piece is issued as
soon as blocks 0-7 of the next layer's projection are done, overlapping the
tail blocks' aggregation; only the small piece sits on the critical path.
p_full row layout is therefore [8 cores x rows 0:1024 | 8 cores x rows
1024:1280] and the gather indices are computed against that layout.

Mean-pool partials via matmul against a node->graph one-hot, AllReduce,
final linear on-chip.
"""

import numpy as np
import ml_dtypes

BF16 = ml_dtypes.bfloat16
E4M3 = ml_dtypes.float8_e4m3

N, E, D, OUT, G = 10000, 160000, 512, 128, 64
NCORES, P = 8, 128
NBLK = 10                     # 128-node blocks per core
NC_NODES = NBLK * P           # 1280
NPAD = NCORES * NC_NODES      # 10240
NLAYERS = 5
KD = D // P                   # 4 chunks of in-channels
HD = D // 2                   # DoubleRow output half-width (256)
AG1_BLKS = 8                  # blocks covered by the early AllGather piece


def _wrap_idx(a):
    """[L] ints -> [128, L//16] int16 SWDGE index layout (16-partition wrap,
    replicated for the 8 Q7 cores)."""
    L = len(a)
    w = a.astype(np.int16).reshape(L // 16, 16).T
    return np.ascontiguousarray(np.tile(w, (8, 1)))


def _balance_nodes(dst):
    """Greedy multiway partition: assign nodes to 80 bins of exactly 128
    slots, minimizing the max per-bin incident-edge count. Returns
    new2old[NPAD] (old node id or -1 for padding)."""
    import heapq

    nbins = NCORES * NBLK
    deg = np.bincount(dst, minlength=N)
    order = np.argsort(-deg, kind="stable")
    cnt = np.zeros(nbins, np.int64)
    edges = np.zeros(nbins, np.int64)
    heap = [(0, b) for b in range(nbins)]
    heapq.heapify(heap)
    node_lists = [[] for _ in range(nbins)]
    for v in order:
        while True:
            _, b = heapq.heappop(heap)
            if cnt[b] < P:
                break
        node_lists[b].append(v)
        cnt[b] += 1
        edges[b] += deg[v]
        if cnt[b] < P:
            heapq.heappush(heap, (edges[b], b))
    new2old = np.full(NPAD, -1, np.int64)
    for b in range(nbins):
        lst = node_lists[b]
        new2old[b * P:b * P + len(lst)] = lst
    return new2old, int(edges.max())


def _ag_split():
    import os
    return bool(int(os.environ.get("GCN_AG_SPLIT", "1")))


def _row_of_new(j):
    """p_full row index for permuted node position j (split-AllGather
    layout: [8 cores x rows 0:1024 | 8 cores x rows 1024:1280])."""
    c, r = j // NC_NODES, j % NC_NODES
    if not _ag_split():
        return c * NC_NODES + r
    lo = AG1_BLKS * P
    return np.where(r < lo, c * lo + r,
                    NCORES * lo + c * (NC_NODES - lo) + (r - lo))


def _prep(inputs):
    x = np.asarray(inputs["x"], np.float32)
    ei = np.asarray(inputs["edge_index"]).astype(np.int64)
    batch = np.asarray(inputs["batch"]).astype(np.int64)
    src, dst = ei[0], ei[1]

    new2old, max_edges = _balance_nodes(dst)
    old2new = np.full(N, -1, np.int64)
    valid = new2old >= 0
    old2new[new2old[valid]] = np.nonzero(valid)[0]

    T = max(2, int(np.ceil(max_edges / P)))
    T += T % 2                     # DoubleRow consumes tile pairs
    L = T * P

    dst_new = old2new[dst]
    src_row = _row_of_new(old2new[src])
    order = np.argsort(dst_new, kind="stable")
    ds_, sr_ = dst_new[order], src_row[order]
    starts = np.searchsorted(ds_, np.arange(0, NPAD + 1, P))

    xp = np.zeros((NPAD, D), np.float32)
    xp[valid] = x[new2old[valid]]
    batch_new = np.full(NPAD, -1, np.int64)
    batch_new[valid] = batch[new2old[valid]]

    counts_g = np.bincount(batch, minlength=G)[:G]
    inv = (1.0 / np.maximum(counts_g, 1.0)).astype(np.float32)

    per_core = []
    for c in range(NCORES):
        idx_blocks = []
        oh_flat = np.zeros((P, NBLK * L), E4M3)
        goh = np.zeros((P, NBLK * G), BF16)
        for b in range(NBLK):
            gb = c * NBLK + b
            lo = gb * P
            s0, s1 = int(starts[gb]), int(starts[gb + 1])
            n = s1 - s0
            assert n <= L
            srcs = np.zeros(L, np.int64)
            srcs[:n] = sr_[s0:s1]
            dloc = ds_[s0:s1] - lo
            oh = np.zeros((L, P), E4M3)
            oh[np.arange(n), dloc] = 1
            idx_blocks.append(_wrap_idx(srcs))
            oh_flat[:, b * L:(b + 1) * L] = (
                oh.reshape(T, P, P).transpose(1, 0, 2).reshape(P, L))
            nodes = lo + np.arange(P)
            bt = batch_new[nodes]
            ok = bt >= 0
            goh[ok, b * G + bt[ok]] = 1

        shard = xp[c * NC_NODES:(c + 1) * NC_NODES].astype(BF16)
        xt0 = np.ascontiguousarray(
            shard.T.reshape(KD, P, NC_NODES).transpose(1, 0, 2))
        per_core.append(dict(
            x_shard=np.ascontiguousarray(shard),
            xt0=xt0,
            ohot=oh_flat,
            idxe=np.ascontiguousarray(np.concatenate(idx_blocks, axis=1)),
            goh=goh,
        ))

    wr = np.zeros((P, NLAYERS, KD, D), BF16)
    ws = np.zeros((P, NLAYERS, KD, D), BF16)
    bias = np.zeros((P, NLAYERS, D), BF16)
    for l in range(NLAYERS):
        wr[:, l] = np.asarray(inputs[f"Wr{l+1}"], np.float32).reshape(
            KD, P, D).transpose(1, 0, 2).astype(BF16)
        ws[:, l] = np.asarray(inputs[f"Ws{l+1}"], np.float32).reshape(
            KD, P, D).transpose(1, 0, 2).astype(BF16)
        bias[0, l] = np.asarray(inputs[f"b{l+1}"], np.float32).astype(BF16)
    ones_e0 = np.zeros((P, P), BF16)
    ones_e0[0, :] = 1
    wlin = np.ascontiguousarray(
        np.asarray(inputs["Wlin"], np.float32).reshape(KD, P, OUT)
        .transpose(1, 0, 2).astype(BF16))
    blin = np.tile(np.asarray(inputs["blin"], np.float32).reshape(OUT, 1),
                   (1, 1)).astype(np.float32)
    shared = dict(
        wr=wr, ws=ws, bias=bias, ones=ones_e0, wlin=wlin, blin=blin,
        invt=np.ascontiguousarray(np.tile(inv, (P, KD)).astype(np.float32)),
        ident=np.eye(P, dtype=BF16),
    )
    return per_core, shared, T


def _unwrap(w, L):
    """inverse of _wrap_idx: [128, L//16] -> [L]"""
    return np.ascontiguousarray(w[:16].T).reshape(-1)[:L].astype(np.int64)


def emulate(inputs):
    """Numpy emulation of the exact device dataflow (bf16/fp8 casts
    included). Validates all host-side index/one-hot bookkeeping."""
    per_core, shared, T = _prep(inputs)
    L = T * P
    f32 = np.float32

    xs = [pc["x_shard"].astype(f32) for pc in per_core]       # [1280, 512]
    for l in range(NLAYERS):
        ws_l = np.concatenate([shared["ws"][:, l, k, :] for k in range(KD)],
                              axis=0).astype(f32)
        wr_l = np.concatenate([shared["wr"][:, l, k, :] for k in range(KD)],
                              axis=0).astype(f32)
        b_l = shared["bias"][0, l].astype(f32)
        # p = x @ Wr, cast fp8, "AllGather" into the p_full row layout
        p_full = np.zeros((NPAD, D), f32)
        for c in range(NCORES):
            p = (xs[c] @ wr_l).astype(E4M3).astype(f32)
            rows = _row_of_new(c * NC_NODES + np.arange(NC_NODES))
            p_full[rows] = p
        new_xs = []
        for c in range(NCORES):
            nx = np.zeros((NC_NODES, D), f32)
            for b in range(NBLK):
                idx = _unwrap(
                    per_core[c]["idxe"][:, b * (L // 16):(b + 1) * (L // 16)], L)
                gath = p_full[idx].astype(E4M3).astype(f32)   # [L, 512]
                acc = np.zeros((P, D), f32)
                for t in range(T):
                    oh = per_core[c]["ohot"][
                        :, (b * T + t) * P:(b * T + t + 1) * P].astype(f32)
                    acc += oh.T @ gath[t * P:(t + 1) * P]
                blk = xs[c][b * P:(b + 1) * P]
                acc += blk @ ws_l + b_l
                val = (acc.astype(f32) + blk)
                if l < NLAYERS - 1:
                    val = np.maximum(val, 0)
                nx[b * P:(b + 1) * P] = val.astype(BF16).astype(f32)
            new_xs.append(nx)
        xs = new_xs
    # pooling
    pooled_T = np.zeros((D, G), f32)
    for c in range(NCORES):
        goh = per_core[c]["goh"].astype(f32)
        for b in range(NBLK):
            blk = xs[c][b * P:(b + 1) * P].astype(BF16).astype(f32)
            for j in range(KD):
                pooled_T[j * P:(j + 1) * P] += (
                    blk[:, j * P:(j + 1) * P].T @ goh[:, b * G:(b + 1) * G])
    inv = shared["invt"][0, :G].astype(f32)
    pooled_T = (pooled_T * inv[None, :]).astype(BF16).astype(f32)
    wlin = np.concatenate([shared["wlin"][:, k, :] for k in range(KD)],
                          axis=0).astype(f32)                 # [512, 128]
    out_T = wlin.T @ pooled_T + shared["blin"][:, :1]         # [128, 64]
    return np.ascontiguousarray(out_T.T).astype(np.float32)


def _build(T, use_bias=None, enable_asserts=False):
    import os
    n_layers = int(os.environ.get("GCN_LAYERS", NLAYERS))
    no_gather = bool(int(os.environ.get("GCN_NO_GATHER", "0")))
    no_cc = bool(int(os.environ.get("GCN_NO_CC", "0")))
    bP = int(os.environ.get("GCN_BANKS", "1"))
    gbufs = int(os.environ.get("GCN_GBUFS", "5"))
    # gather tiles per DMA: SWDGE dispatch is 994ns fixed + 0.34ns/row, so
    # fewer, larger gathers win; 16 = one gather per 128-node block
    gchunk = int(os.environ.get("GCN_GCHUNK", "16"))
    # blocks of root-matmul lookahead: PE work queued ahead of the first
    # gather-dependent matmul, hiding the AllGather at each layer boundary
    lookR = int(os.environ.get("GCN_LOOKAHEAD", "4"))
    if use_bias is None:
        use_bias = [True] * NLAYERS
    import concourse.bass as bass
    import concourse.mybir as mybir
    import concourse.tile as tile
    from concourse import bacc

    F32 = mybir.dt.float32
    BF = mybir.dt.bfloat16
    FP8 = mybir.dt.float8e4
    I16 = mybir.dt.int16
    ADD = mybir.AluOpType.add
    MUL = mybir.AluOpType.mult
    DR = mybir.MatmulPerfMode.DoubleRow
    L = T * P
    LO = AG1_BLKS * P              # 1024 rows in the early AllGather piece
    HI = NC_NODES - LO             # 256 rows in the late piece
    RG = [list(range(NCORES))]

    nc = bacc.Bacc("TRN2", target_bir_lowering=False, debug=False,
                   enable_asserts=enable_asserts, num_devices=NCORES)

    # per-core inputs
    x_d = nc.dram_tensor("x_shard", [NC_NODES, D], BF, kind="ExternalInput")
    xt0_d = nc.dram_tensor("xt0", [P, KD, NC_NODES], BF, kind="ExternalInput")
    oh_d = nc.dram_tensor("ohot", [P, NBLK * L], FP8, kind="ExternalInput")
    idxe_d = nc.dram_tensor("idxe", [P, NBLK * (L // 16)], I16, kind="ExternalInput")
    goh_d = nc.dram_tensor("goh", [P, NBLK * G], BF, kind="ExternalInput")
    # shared inputs
    wr_d = nc.dram_tensor("wr", [P, NLAYERS, KD, D], BF, kind="ExternalInput")
    ws_d = nc.dram_tensor("ws", [P, NLAYERS, KD, D], BF, kind="ExternalInput")
    bias_d = nc.dram_tensor("bias", [P, NLAYERS, D], BF, kind="ExternalInput")
    ones_d = nc.dram_tensor("ones", [P, P], BF, kind="ExternalInput")
    wlin_d = nc.dram_tensor("wlin", [P, KD, OUT], BF, kind="ExternalInput")
    blin_d = nc.dram_tensor("blin", [OUT, 1], F32, kind="ExternalInput")
    invt_d = nc.dram_tensor("invt", [P, KD * G], F32, kind="ExternalInput")
    ident_d = nc.dram_tensor("ident", [P, P], BF, kind="ExternalInput")
    # internal DRAM (double-buffered by layer parity so the AllGather for
    # layer l+1 never WAR-depends on layer l's gathers)
    p_shard = [nc.dram_tensor(f"p_shard{i}", [NC_NODES, D], FP8) for i in (0, 1)]
    p_full = [nc.dram_tensor(f"p_full{i}", [NPAD, D], FP8, addr_space="Shared")
              for i in (0, 1)]
    fin_in = nc.dram_tensor("fin_in", [OUT, G], F32)
    fin_out = nc.dram_tensor("fin_out", [OUT, G], F32, addr_space="Shared")
    # output
    out_d = nc.dram_tensor("out_t", [OUT, G], F32, kind="ExternalOutput")

    with tile.TileContext(nc) as tc:
        with (
            tc.tile_pool(name="const", bufs=1) as const,
            tc.tile_pool(name="xs", bufs=2) as xpool,
            tc.tile_pool(name="xt", bufs=2) as xtpool,
            tc.tile_pool(name="gath", bufs=gbufs) as gpool,
            tc.tile_pool(name="small", bufs=int(os.environ.get("GCN_SBUFS", "4"))) as spool,
            tc.tile_pool(name="psP", bufs=bP, space="PSUM") as psP,
            tc.tile_pool(name="psA", bufs=max(1, lookR), space="PSUM") as psA,
            tc.tile_pool(name="psS", bufs=1, space="PSUM") as psS,
            tc.tile_pool(name="psT", bufs=2, space="PSUM") as psT,
        ):
            # ---- constants to SBUF. DMA order is critical-path order: the
            # layer-0 projection needs only xt0 + Wr[0], so those go first
            # (PE starts ~6us in instead of waiting ~30us for everything);
            # gather tables (idxe/oh) next; remaining layers' weights follow.
            xt_cur = xtpool.tile([P, KD, NC_NODES], BF, tag="xt")
            nc.sync.dma_start(xt_cur[:], xt0_d[:])
            wr_sb = const.tile([P, NLAYERS, KD, D], BF, tag="wr")
            nc.sync.dma_start(wr_sb[:, 0], wr_d[:, 0])
            ws_sb = const.tile([P, NLAYERS, KD, D], BF, tag="ws")
            nc.sync.dma_start(ws_sb[:, 0], ws_d[:, 0])
            bias_sb = const.tile([P, NLAYERS, D], BF, tag="bias")
            nc.sync.dma_start(bias_sb[:, 0], bias_d[:, 0])
            ones_sb = const.tile([P, P], BF, tag="ones")
            nc.sync.dma_start(ones_sb[:], ones_d[:])
            xs_cur = xpool.tile([P, NBLK, D], BF, tag="xs")
            nc.sync.dma_start(xs_cur[:], x_d.ap().rearrange("(b p) d -> p b d", p=P))
            idxe_sb = const.tile([P, NBLK * (L // 16)], I16, tag="idxe")
            nc.sync.dma_start(idxe_sb[:], idxe_d[:])
            # deferred constants ride the (otherwise idle) ACT engine's DMA
            # queue so the SP queue stays clear for the prologue's p_sb
            # writes + p_full copy — the critical chain to the first gather
            oh_sb = const.tile([P, NBLK * L], FP8, tag="oh")
            nc.scalar.dma_start(oh_sb[:], oh_d[:])
            ident_sb = const.tile([P, P], BF, tag="ident")
            nc.scalar.dma_start(ident_sb[:], ident_d[:])
            for ll in range(1, NLAYERS):
                nc.scalar.dma_start(wr_sb[:, ll], wr_d[:, ll])
                nc.scalar.dma_start(ws_sb[:, ll], ws_d[:, ll])
                nc.scalar.dma_start(bias_sb[:, ll], bias_d[:, ll])
            goh_sb = const.tile([P, NBLK * G], BF, tag="goh")
            nc.scalar.dma_start(goh_sb[:], goh_d[:])
            wlin_sb = const.tile([P, KD, OUT], BF, tag="wlin")
            nc.scalar.dma_start(wlin_sb[:], wlin_d[:])
            blin_sb = const.tile([OUT, 1], F32, tag="blin")
            nc.scalar.dma_start(blin_sb[:], blin_d[:])
            invt_sb = const.tile([P, KD * G], F32, tag="invt")
            nc.scalar.dma_start(invt_sb[:], invt_d[:])

            def emit_p_block(xt_src, layer, m, pbuf):
                """p[l=layer] block m = x_l[block m] @ Wr_l, into p_shard[pbuf]."""
                pps = psP.tile([P, D], F32, tag="pps", name=f"pps_{layer}_{m}")
                for k in range(KD):
                    nc.tensor.matmul(
                        pps[:],
                        lhsT=xt_src[:, k, m * P:(m + 1) * P],
                        rhs=wr_sb[:, layer, k, :],
                        start=(k == 0), stop=(k == KD - 1))
                p_sb = spool.tile([P, D], FP8, tag="psb", name=f"psb_{layer}_{m}")
                nc.vector.tensor_copy(p_sb[:], pps[:])
                nc.sync.dma_start(
                    p_shard[pbuf][m * P:(m + 1) * P, :], p_sb[:])

            ag_split = _ag_split()

            def emit_ag1(pbuf):
                if not ag_split:
                    return
                if no_cc:
                    nc.sync.dma_start(
                        p_full[pbuf][:LO, :], p_shard[pbuf][:LO, :])
                else:
                    nc.gpsimd.collective_compute(
                        "AllGather", mybir.AluOpType.bypass, replica_groups=RG,
                        ins=[p_shard[pbuf][:LO, :]],
                        outs=[p_full[pbuf][:NCORES * LO, :]])

            def emit_ag2(pbuf):
                if not ag_split:
                    if no_cc:
                        nc.sync.dma_start(
                            p_full[pbuf][:NC_NODES, :], p_shard[pbuf][:])
                    else:
                        nc.gpsimd.collective_compute(
                            "AllGather", mybir.AluOpType.bypass,
                            replica_groups=RG,
                            ins=[p_shard[pbuf][:]], outs=[p_full[pbuf][:]])
                    return
                if no_cc:
                    nc.sync.dma_start(
                        p_full[pbuf][NCORES * LO:NCORES * LO + HI, :],
                        p_shard[pbuf][LO:, :])
                else:
                    nc.gpsimd.collective_compute(
                        "AllGather", mybir.AluOpType.bypass, replica_groups=RG,
                        ins=[p_shard[pbuf][LO:, :]],
                        outs=[p_full[pbuf][NCORES * LO:, :]])

            # prologue: projection for layer 0
            for m in range(NBLK):
                emit_p_block(xt_cur, 0, m, 0)
            emit_ag1(0)
            emit_ag2(0)

            # one PSUM bank holds all four 64-col pool-partial accumulation
            # groups plus the final-linear group (5 x 256B sub-bank regions).
            # start=True would zero the whole 2KB zero-region (clobbering the
            # sibling groups), so the bank is memset once and every matmul
            # into it accumulates with start=False.
            pool_bank = psS.tile([P, (KD + 1) * G], F32, tag="poolbank",
                                 name="pool_bank")
            nc.vector.memset(pool_bank[:], 0)
            pool_ps = [pool_bank[:, j * G:(j + 1) * G] for j in range(KD)]
            for l in range(n_layers):
                pbuf = l % 2
                xs_next = xpool.tile([P, NBLK, D], BF, tag="xs")
                last = l == NLAYERS - 1
                if not last:
                    xt_next = xtpool.tile([P, KD, NC_NODES], BF, tag="xt")

                aps_blk = [None] * NBLK

                def emit_root(bb, l=l, aps_blk=aps_blk, xt_cur=xt_cur):
                    """Root x@Ws (+bias) for block bb — needs only resident
                    data, so it can run on PE while the AllGather is in
                    flight."""
                    aps = psA.tile([P, D], F32, tag="aps",
                                   name=f"aps_{l}_{bb}")
                    for k in range(KD):
                        nc.tensor.matmul(
                            aps[:],
                            lhsT=xt_cur[:, k, bb * P:(bb + 1) * P],
                            rhs=ws_sb[:, l, k, :],
                            start=(k == 0), stop=False)
                    if use_bias[l]:
                        nc.tensor.matmul(
                            aps[:], lhsT=ones_sb[:], rhs=bias_sb[:, l, :],
                            start=False, stop=False)
                    aps_blk[bb] = aps

                for j in range(min(lookR, NBLK)):
                    emit_root(j)
                for b in range(NBLK):
                    if b > 0 and b + lookR - 1 < NBLK:
                        emit_root(b + lookR - 1)
                    g = gpool.tile([P, T, D], FP8, tag="g")
                    if no_gather:
                        nc.vector.memset(g[:], 0)
                    else:
                        # split the block gather so segment matmuls on early
                        # tiles overlap later chunks' DMA drain; chunks stay
                        # even so DoubleRow pairs never straddle a chunk
                        col0 = b * (L // 16)
                        # block 0 restarts PE right after the AllGather, so
                        # split its gather for earlier first-tile arrival
                        gc = gchunk if b > 0 else max(2, gchunk // 2)
                        for s0 in range(0, T, gc):
                            s1 = min(s0 + gc, T)
                            nc.gpsimd.dma_gather(
                                g[:, s0:s1, :], p_full[pbuf][:],
                                idxe_sb[:, col0 + s0 * 8:col0 + s1 * 8],
                                (s1 - s0) * P, (s1 - s0) * P, D,
                                single_packet=False)
                    aps = aps_blk[b]
                    # fp8 DoubleRow segment matmuls: each consumes a pair of
                    # 128-edge tiles (K=256) and one 256-wide output half
                    for t in range(0, T, 2):
                        oh_pair = oh_sb[
                            :, (b * T + t) * P:(b * T + t + 2) * P
                        ].rearrange("p (two m) -> p two m", two=2)
                        for h in range(2):
                            nc.tensor.matmul(
                                aps[:, h * HD:(h + 1) * HD],
                                lhsT=oh_pair,
                                rhs=g[:, t:t + 2, h * HD:(h + 1) * HD],
                                start=False,
                                stop=(t + 2 >= T),
                                perf_mode=DR,
                                skip_group_check=True)
                    if last:
                        nc.vector.tensor_tensor(
                            xs_next[:, b, :], aps[:], xs_cur[:, b, :], op=ADD)
                        # pooling partials for this block, interleaved so they
                        # hide under later blocks' gathers
                        for j in range(KD):
                            nc.tensor.matmul(
                                pool_ps[j],
                                lhsT=xs_next[:, b, j * P:(j + 1) * P],
                                rhs=goh_sb[:, b * G:(b + 1) * G],
                                start=False, stop=(b == NBLK - 1),
                                skip_group_check=True)
                    else:
                        t1 = spool.tile([P, D], BF, tag="t1")
                        nc.vector.tensor_tensor(
                            t1[:], aps[:], xs_cur[:, b, :], op=ADD)
                        nc.scalar.activation(
                            xs_next[:, b, :], t1[:],
                            func=mybir.ActivationFunctionType.Relu)
                        # transpose new block into xt_next (channel-major);
                        # the PSUM->SBUF copies ride the Pool engine, which
                        # is otherwise idle (DVE carries the residual adds)
                        for j in range(KD):
                            trps = psT.tile([P, P], BF, tag="tr")
                            nc.tensor.transpose(
                                trps[:], xs_next[:, b, j * P:(j + 1) * P],
                                ident_sb[:])
                            nc.vector.tensor_copy(
                                xt_next[:, j, b * P:(b + 1) * P], trps[:])
                        # pipelined projection for layer l+1, block b
                        emit_p_block(xt_next, l + 1, b, 1 - pbuf)
                        if b == AG1_BLKS - 1:
                            emit_ag1(1 - pbuf)
                if not last:
                    emit_ag2(1 - pbuf)
                    xt_cur = xt_next
                xs_cur = xs_next

            # ---- pooling partials were accumulated inside the last layer's
            # block loop (one PSUM bank per 128-channel chunk). The final
            # linear is applied per-core BEFORE the AllReduce (everything is
            # linear, inv is a per-graph diagonal), shrinking the collective
            # from [P, KD*G] f32 (128KB) to [OUT, G] f32 (32KB).
            poolbf = spool.tile([P, KD * G], BF, tag="poolbf")
            for j in range(KD):
                nc.vector.tensor_tensor(
                    poolbf[:, j * G:(j + 1) * G], pool_ps[j],
                    invt_sb[:, j * G:(j + 1) * G], op=MUL)
            fin_ps = pool_bank[:, KD * G:(KD + 1) * G]
            for k in range(KD):
                nc.tensor.matmul(
                    fin_ps, lhsT=wlin_sb[:, k, :],
                    rhs=poolbf[:, k * G:(k + 1) * G],
                    start=False, stop=(k == KD - 1),
                    skip_group_check=True)
            fin_sb = spool.tile([OUT, G], F32, tag="fin_sb")
            nc.vector.tensor_copy(fin_sb[:], fin_ps)
            nc.sync.dma_start(fin_in[:], fin_sb[:])
            if no_cc:
                nc.sync.dma_start(fin_out[:], fin_in[:])
            else:
                nc.gpsimd.collective_compute(
                    "AllReduce", ADD, replica_groups=RG,
                    ins=[fin_in[:]], outs=[fin_out[:]])
            red_sb = spool.tile([OUT, G], F32, tag="red_sb")
            nc.sync.dma_start(red_sb[:], fin_out[:])
            out_sb = spool.tile([OUT, G], F32, tag="out_sb")
            nc.vector.tensor_tensor(
                out_sb[:], red_sb[:], blin_sb[:, :1].to_broadcast([OUT, G]),
                op=ADD)
            nc.sync.dma_start(out_d[:], out_sb[:])

    nc.compile()
    return nc


def kernel(**inputs):
    import os
    from concourse.bass_utils import run_bass_kernel_spmd

    per_core, shared, T = _prep(inputs)
    use_bias = [bool(np.asarray(inputs[f"b{l+1}"]).astype(np.float32).any())
                for l in range(NLAYERS)]
    nc = _build(T, use_bias)
    in_maps = [{**pc, **shared} for pc in per_core]
    trace = bool(int(os.environ.get("GCN_TRACE", "0")))
    res = run_bass_kernel_spmd(nc, in_maps, core_ids=list(range(NCORES)),
                               trace=trace)
    if trace:
        print(f"HW exec time: {res.exec_time_ns} ns")
        if res.instructions_and_trace is not None:
            print("trace:", res.instructions_and_trace[1])
    out_t = res.results[0]["out_t"]
    return np.ascontiguousarray(out_t.T).astype(np.float32)


# revision 34
# speedup vs baseline: 1.5858x; 1.0001x over previous
"""Distributed GCN (5x GraphConv(add) + residual/ReLU + mean-pool + linear)
for 8 Trainium2 NeuronCores.

Sharding: nodes are permuted so every 128-node block has a near-equal number
of incident (destination) edges, then partitioned contiguously across cores
(1280 nodes/core). Each core owns the edges whose destination lands in its
shard. Aggregation is computed as A@(x@Wr): project first (p = x@Wr),
AllGather p, gather p[src[e]] rows with SWDGE dma_gather, then reduce edge
tiles onto destination nodes with one-hot segment matmuls on the tensor
engine.

The aggregation path runs in fp8 e4m3: p is stored/AllGathered/gathered as
fp8 (|p| < 240 for this model scale) and the one-hot segment matmuls use
DoubleRow perf mode (K=256 contraction, 0.5 PE cycles/row = 2x bf16
throughput). Edge balancing makes T (128-edge tiles per block) uniform and
even, which DoubleRow pairing requires. x@Ws + bias accumulate into the same
PSUM bank; residual+ReLU on DVE/ACT. Weights and node features stay bf16.

The AllGather is split 1024+256 rows per core: the big piece is issued as
soon as blocks 0-7 of the next layer's projection are done, overlapping the
tail blocks' aggregation; only the small piece sits on the critical path.
p_full row layout is therefore [8 cores x rows 0:1024 | 8 cores x rows
1024:1280] and the gather indices are computed against that layout. At each
layer boundary, a lookahead of root (x@Ws) matmuls — which need no gathered
data — keeps the PE fed while the AllGather completes.

SWDGE gather dispatch costs ~1us fixed per instruction, so each 128-node
block's 2048 edge rows are fetched in one dma_gather (two for block 0, which
sits right after the AllGather on the critical path). Deferred constant
loads ride the ACT engine's DMA queue so the SP queue stays clear for the
prologue's projection writes.

Mean-pool partials via matmul against a node->graph one-hot; the final
linear is applied per-core before a 32KB AllReduce (valid because mean-pool
and the linear commute with the cross-core sum). All five pool/final PSUM
accumulation groups share one memset bank with start=False accumulation —
start=True would zero the whole 2KB zero-region and clobber the siblings.
"""

import numpy as np
import ml_dtypes

BF16 = ml_dtypes.bfloat16
E4M3 = ml_dtypes.float8_e4m3

N, E, D, OUT, G = 10000, 160000, 512, 128, 64
NCORES, P = 8, 128
NBLK = 10                     # 128-node blocks per core
NC_NODES = NBLK * P           # 1280
NPAD = NCORES * NC_NODES      # 10240
NLAYERS = 5
KD = D // P                   # 4 chunks of in-channels
HD = D // 2                   # DoubleRow output half-width (256)
AG1_BLKS = 8                  # blocks covered by the early AllGather piece


def _wrap_idx(a):
    """[L] ints -> [128, L//16] int16 SWDGE index layout (16-partition wrap,
    replicated for the 8 Q7 cores)."""
    L = len(a)
    w = a.astype(np.int16).reshape(L // 16, 16).T
    return np.ascontiguousarray(np.tile(w, (8, 1)))


def _balance_nodes(dst):
    """Greedy multiway partition: assign nodes to 80 bins of exactly 128
    slots, minimizing the max per-bin incident-edge count. Returns
    new2old[NPAD] (old node id or -1 for padding)."""
    import heapq

    nbins = NCORES * NBLK
    deg = np.bincount(dst, minlength=N)
    order = np.argsort(-deg, kind="stable")
    cnt = np.zeros(nbins, np.int64)
    edges = np.zeros(nbins, np.int64)
    heap = [(0, b) for b in range(nbins)]
    heapq.heapify(heap)
    node_lists = [[] for _ in range(nbins)]
    for v in order:
        while True:
            _, b = heapq.heappop(heap)
            if cnt[b] < P:
                break
        node_lists[b].append(v)
        cnt[b] += 1
        edges[b] += deg[v]
        if cnt[b] < P:
            heapq.heappush(heap, (edges[b], b))
    new2old = np.full(NPAD, -1, np.int64)
    for b in range(nbins):
        lst = node_lists[b]
        new2old[b * P:b * P + len(lst)] = lst
    return new2old, int(edges.max())


def _ag_split():
    import os
    return bool(int(os.environ.get("GCN_AG_SPLIT", "1")))


def _row_of_new(j):
    """p_full row index for permuted node position j (split-AllGather
    layout: [8 cores x rows 0:1024 | 8 cores x rows 1024:1280])."""
    c, r = j // NC_NODES, j % NC_NODES
    if not _ag_split():
        return c * NC_NODES + r
    lo = AG1_BLKS * P
    return np.where(r < lo, c * lo + r,
                    NCORES * lo + c * (NC_NODES - lo) + (r - lo))


def _prep(inputs):
    x = np.asarray(inputs["x"], np.float32)
    ei = np.asarray(inputs["edge_index"]).astype(np.int64)
    batch = np.asarray(inputs["batch"]).astype(np.int64)
    src, dst = ei[0], ei[1]

    new2old, max_edges = _balance_nodes(dst)
    old2new = np.full(N, -1, np.int64)
    valid = new2old >= 0
    old2new[new2old[valid]] = np.nonzero(valid)[0]

    T = max(2, int(np.ceil(max_edges / P)))
    T += T % 2                     # DoubleRow consumes tile pairs
    L = T * P

    dst_new = old2new[dst]
    src_row = _row_of_new(old2new[src])
    order = np.argsort(dst_new, kind="stable")
    ds_, sr_ = dst_new[order], src_row[order]
    starts = np.searchsorted(ds_, np.arange(0, NPAD + 1, P))

    xp = np.zeros((NPAD, D), np.float32)
    xp[valid] = x[new2old[valid]]
    batch_new = np.full(NPAD, -1, np.int64)
    batch_new[valid] = batch[new2old[valid]]

    counts_g = np.bincount(batch, minlength=G)[:G]
    inv = (1.0 / np.maximum(counts_g, 1.0)).astype(np.float32)

    per_core = []
    for c in range(NCORES):
        idx_blocks = []
        oh_flat = np.zeros((P, NBLK * L), E4M3)
        goh = np.zeros((P, NBLK * G), BF16)
        for b in range(NBLK):
            gb = c * NBLK + b
            lo = gb * P
            s0, s1 = int(starts[gb]), int(starts[gb + 1])
            n = s1 - s0
            assert n <= L
            srcs = np.zeros(L, np.int64)
            srcs[:n] = sr_[s0:s1]
            dloc = ds_[s0:s1] - lo
            oh = np.zeros((L, P), E4M3)
            oh[np.arange(n), dloc] = 1
            idx_blocks.append(_wrap_idx(srcs))
            oh_flat[:, b * L:(b + 1) * L] = (
                oh.reshape(T, P, P).transpose(1, 0, 2).reshape(P, L))
            nodes = lo + np.arange(P)
            bt = batch_new[nodes]
            ok = bt >= 0
            goh[ok, b * G + bt[ok]] = 1

        shard = xp[c * NC_NODES:(c + 1) * NC_NODES].astype(BF16)
        xt0 = np.ascontiguousarray(
            shard.T.reshape(KD, P, NC_NODES).transpose(1, 0, 2))
        per_core.append(dict(
            x_shard=np.ascontiguousarray(shard),
            xt0=xt0,
            ohot=oh_flat,
            idxe=np.ascontiguousarray(np.concatenate(idx_blocks, axis=1)),
            goh=goh,
        ))

    wr = np.zeros((P, NLAYERS, KD, D), BF16)
    ws = np.zeros((P, NLAYERS, KD, D), BF16)
    bias = np.zeros((P, NLAYERS, D), BF16)
    for l in range(NLAYERS):
        wr[:, l] = np.asarray(inputs[f"Wr{l+1}"], np.float32).reshape(
            KD, P, D).transpose(1, 0, 2).astype(BF16)
        ws[:, l] = np.asarray(inputs[f"Ws{l+1}"], np.float32).reshape(
            KD, P, D).transpose(1, 0, 2).astype(BF16)
        bias[0, l] = np.asarray(inputs[f"b{l+1}"], np.float32).astype(BF16)
    ones_e0 = np.zeros((P, P), BF16)
    ones_e0[0, :] = 1
    wlin = np.ascontiguousarray(
        np.asarray(inputs["Wlin"], np.float32).reshape(KD, P, OUT)
        .transpose(1, 0, 2).astype(BF16))
    blin = np.tile(np.asarray(inputs["blin"], np.float32).reshape(OUT, 1),
                   (1, 1)).astype(np.float32)
    shared = dict(
        wr=wr, ws=ws, bias=bias, ones=ones_e0, wlin=wlin, blin=blin,
        invt=np.ascontiguousarray(np.tile(inv, (P, KD)).astype(np.float32)),
        ident=np.eye(P, dtype=BF16),
    )
    return per_core, shared, T


def _unwrap(w, L):
    """inverse of _wrap_idx: [128, L//16] -> [L]"""
    return np.ascontiguousarray(w[:16].T).reshape(-1)[:L].astype(np.int64)


def emulate(inputs):
    """Numpy emulation of the exact device dataflow (bf16/fp8 casts
    included). Validates all host-side index/one-hot bookkeeping."""
    per_core, shared, T = _prep(inputs)
    L = T * P
    f32 = np.float32

    xs = [pc["x_shard"].astype(f32) for pc in per_core]       # [1280, 512]
    for l in range(NLAYERS):
        ws_l = np.concatenate([shared["ws"][:, l, k, :] for k in range(KD)],
                              axis=0).astype(f32)
        wr_l = np.concatenate([shared["wr"][:, l, k, :] for k in range(KD)],
                              axis=0).astype(f32)
        b_l = shared["bias"][0, l].astype(f32)
        # p = x @ Wr, cast fp8, "AllGather" into the p_full row layout
        p_full = np.zeros((NPAD, D), f32)
        for c in range(NCORES):
            p = (xs[c] @ wr_l).astype(E4M3).astype(f32)
            rows = _row_of_new(c * NC_NODES + np.arange(NC_NODES))
            p_full[rows] = p
        new_xs = []
        for c in range(NCORES):
            nx = np.zeros((NC_NODES, D), f32)
            for b in range(NBLK):
                idx = _unwrap(
                    per_core[c]["idxe"][:, b * (L // 16):(b + 1) * (L // 16)], L)
                gath = p_full[idx].astype(E4M3).astype(f32)   # [L, 512]
                acc = np.zeros((P, D), f32)
                for t in range(T):
                    oh = per_core[c]["ohot"][
                        :, (b * T + t) * P:(b * T + t + 1) * P].astype(f32)
                    acc += oh.T @ gath[t * P:(t + 1) * P]
                blk = xs[c][b * P:(b + 1) * P]
                acc += blk @ ws_l + b_l
                val = (acc.astype(f32) + blk)
                if l < NLAYERS - 1:
                    val = np.maximum(val, 0)
                nx[b * P:(b + 1) * P] = val.astype(BF16).astype(f32)
            new_xs.append(nx)
        xs = new_xs
    # pooling
    pooled_T = np.zeros((D, G), f32)
    for c in range(NCORES):
        goh = per_core[c]["goh"].astype(f32)
        for b in range(NBLK):
            blk = xs[c][b * P:(b + 1) * P].astype(BF16).astype(f32)
            for j in range(KD):
                pooled_T[j * P:(j + 1) * P] += (
                    blk[:, j * P:(j + 1) * P].T @ goh[:, b * G:(b + 1) * G])
    inv = shared["invt"][0, :G].astype(f32)
    pooled_T = (pooled_T * inv[None, :]).astype(BF16).astype(f32)
    wlin = np.concatenate([shared["wlin"][:, k, :] for k in range(KD)],
                          axis=0).astype(f32)                 # [512, 128]
    out_T = wlin.T @ pooled_T + shared["blin"][:, :1]         # [128, 64]
    return np.ascontiguousarray(out_T.T).astype(np.float32)


def _build(T, use_bias=None, enable_asserts=False):
    import os
    n_layers = int(os.environ.get("GCN_LAYERS", NLAYERS))
    no_gather = bool(int(os.environ.get("GCN_NO_GATHER", "0")))
    no_cc = bool(int(os.environ.get("GCN_NO_CC", "0")))
    bP = int(os.environ.get("GCN_BANKS", "1"))
    gbufs = int(os.environ.get("GCN_GBUFS", "5"))
    # gather tiles per DMA: SWDGE dispatch is 994ns fixed + 0.34ns/row, so
    # fewer, larger gathers win; 16 = one gather per 128-node block
    gchunk = int(os.environ.get("GCN_GCHUNK", "16"))
    # blocks of root-matmul lookahead: PE work queued ahead of the first
    # gather-dependent matmul, hiding the AllGather at each layer boundary
    lookR = int(os.environ.get("GCN_LOOKAHEAD", "4"))
    if use_bias is None:
        use_bias = [True] * NLAYERS
    import concourse.bass as bass
    import concourse.mybir as mybir
    import concourse.tile as tile
    from concourse import bacc

    F32 = mybir.dt.float32
    BF = mybir.dt.bfloat16
    FP8 = mybir.dt.float8e4
    I16 = mybir.dt.int16
    ADD = mybir.AluOpType.add
    MUL = mybir.AluOpType.mult
    DR = mybir.MatmulPerfMode.DoubleRow
    L = T * P
    LO = AG1_BLKS * P              # 1024 rows in the early AllGather piece
    HI = NC_NODES - LO             # 256 rows in the late piece
    RG = [list(range(NCORES))]

    nc = bacc.Bacc("TRN2", target_bir_lowering=False, debug=False,
                   enable_asserts=enable_asserts, num_devices=NCORES)

    # per-core inputs
    x_d = nc.dram_tensor("x_shard", [NC_NODES, D], BF, kind="ExternalInput")
    xt0_d = nc.dram_tensor("xt0", [P, KD, NC_NODES], BF, kind="ExternalInput")
    oh_d = nc.dram_tensor("ohot", [P, NBLK * L], FP8, kind="ExternalInput")
    idxe_d = nc.dram_tensor("idxe", [P, NBLK * (L // 16)], I16, kind="ExternalInput")
    goh_d = nc.dram_tensor("goh", [P, NBLK * G], BF, kind="ExternalInput")
    # shared inputs
    wr_d = nc.dram_tensor("wr", [P, NLAYERS, KD, D], BF, kind="ExternalInput")
    ws_d = nc.dram_tensor("ws", [P, NLAYERS, KD, D], BF, kind="ExternalInput")
    bias_d = nc.dram_tensor("bias", [P, NLAYERS, D], BF, kind="ExternalInput")
    ones_d = nc.dram_tensor("ones", [P, P], BF, kind="ExternalInput")
    wlin_d = nc.dram_tensor("wlin", [P, KD, OUT], BF, kind="ExternalInput")
    blin_d = nc.dram_tensor("blin", [OUT, 1], F32, kind="ExternalInput")
    invt_d = nc.dram_tensor("invt", [P, KD * G], F32, kind="ExternalInput")
    ident_d = nc.dram_tensor("ident", [P, P], BF, kind="ExternalInput")
    # internal DRAM (double-buffered by layer parity so the AllGather for
    # layer l+1 never WAR-depends on layer l's gathers)
    p_shard = [nc.dram_tensor(f"p_shard{i}", [NC_NODES, D], FP8) for i in (0, 1)]
    p_full = [nc.dram_tensor(f"p_full{i}", [NPAD, D], FP8, addr_space="Shared")
              for i in (0, 1)]
    fin_in = nc.dram_tensor("fin_in", [OUT, G], F32)
    fin_out = nc.dram_tensor("fin_out", [OUT, G], F32, addr_space="Shared")
    # output
    out_d = nc.dram_tensor("out_t", [OUT, G], F32, kind="ExternalOutput")

    with tile.TileContext(nc) as tc:
        with (
            tc.tile_pool(name="const", bufs=1) as const,
            tc.tile_pool(name="xs", bufs=2) as xpool,
            tc.tile_pool(name="xt", bufs=2) as xtpool,
            tc.tile_pool(name="gath", bufs=gbufs) as gpool,
            tc.tile_pool(name="small", bufs=int(os.environ.get("GCN_SBUFS", "4"))) as spool,
            tc.tile_pool(name="psP", bufs=bP, space="PSUM") as psP,
            tc.tile_pool(name="psA", bufs=max(1, lookR), space="PSUM") as psA,
            tc.tile_pool(name="psS", bufs=1, space="PSUM") as psS,
            tc.tile_pool(name="psT", bufs=2, space="PSUM") as psT,
        ):
            # ---- constants to SBUF. DMA order is critical-path order: the
            # layer-0 projection needs only xt0 + Wr[0], so those go first
            # (PE starts ~6us in instead of waiting ~30us for everything);
            # gather tables (idxe/oh) next; remaining layers' weights follow.
            xt_cur = xtpool.tile([P, KD, NC_NODES], BF, tag="xt")
            nc.sync.dma_start(xt_cur[:], xt0_d[:])
            wr_sb = const.tile([P, NLAYERS, KD, D], BF, tag="wr")
            nc.sync.dma_start(wr_sb[:, 0], wr_d[:, 0])
            ws_sb = const.tile([P, NLAYERS, KD, D], BF, tag="ws")
            nc.sync.dma_start(ws_sb[:, 0], ws_d[:, 0])
            bias_sb = const.tile([P, NLAYERS, D], BF, tag="bias")
            nc.sync.dma_start(bias_sb[:, 0], bias_d[:, 0])
            ones_sb = const.tile([P, P], BF, tag="ones")
            nc.sync.dma_start(ones_sb[:], ones_d[:])
            xs_cur = xpool.tile([P, NBLK, D], BF, tag="xs")
            nc.sync.dma_start(xs_cur[:], x_d.ap().rearrange("(b p) d -> p b d", p=P))
            idxe_sb = const.tile([P, NBLK * (L // 16)], I16, tag="idxe")
            nc.sync.dma_start(idxe_sb[:], idxe_d[:])
            # deferred constants ride the (otherwise idle) ACT engine's DMA
            # queue so the SP queue stays clear for the prologue's p_sb
            # writes + p_full copy — the critical chain to the first gather
            oh_sb = const.tile([P, NBLK * L], FP8, tag="oh")
            nc.scalar.dma_start(oh_sb[:], oh_d[:])
            ident_sb = const.tile([P, P], BF, tag="ident")
            nc.scalar.dma_start(ident_sb[:], ident_d[:])
            for ll in range(1, NLAYERS):
                nc.scalar.dma_start(wr_sb[:, ll], wr_d[:, ll])
                nc.scalar.dma_start(ws_sb[:, ll], ws_d[:, ll])
                nc.scalar.dma_start(bias_sb[:, ll], bias_d[:, ll])
            goh_sb = const.tile([P, NBLK * G], BF, tag="goh")
            nc.scalar.dma_start(goh_sb[:], goh_d[:])
            wlin_sb = const.tile([P, KD, OUT], BF, tag="wlin")
            nc.scalar.dma_start(wlin_sb[:], wlin_d[:])
            blin_sb = const.tile([OUT, 1], F32, tag="blin")
            nc.scalar.dma_start(blin_sb[:], blin_d[:])
            invt_sb = const.tile([P, KD * G], F32, tag="invt")
            nc.scalar.dma_start(invt_sb[:], invt_d[:])

            def emit_p_block(xt_src, layer, m, pbuf):
                """p[l=layer] block m = x_l[block m] @ Wr_l, into p_shard[pbuf]."""
                pps = psP.tile([P, D], F32, tag="pps", name=f"pps_{layer}_{m}")
                for k in range(KD):
                    nc.tensor.matmul(
                        pps[:],
                        lhsT=xt_src[:, k, m * P:(m + 1) * P],
                        rhs=wr_sb[:, layer, k, :],
                        start=(k == 0), stop=(k == KD - 1))
                p_sb = spool.tile([P, D], FP8, tag="psb", name=f"psb_{layer}_{m}")
                nc.vector.tensor_copy(p_sb[:], pps[:])
                nc.sync.dma_start(
                    p_shard[pbuf][m * P:(m + 1) * P, :], p_sb[:])

            ag_split = _ag_split()

            def emit_ag1(pbuf):
                if not ag_split:
                    return
                if no_cc:
                    nc.sync.dma_start(
                        p_full[pbuf][:LO, :], p_shard[pbuf][:LO, :])
                else:
                    nc.gpsimd.collective_compute(
                        "AllGather", mybir.AluOpType.bypass, replica_groups=RG,
                        ins=[p_shard[pbuf][:LO, :]],
                        outs=[p_full[pbuf][:NCORES * LO, :]])

            def emit_ag2(pbuf):
                if not ag_split:
                    if no_cc:
                        nc.sync.dma_start(
                            p_full[pbuf][:NC_NODES, :], p_shard[pbuf][:])
                    else:
                        nc.gpsimd.collective_compute(
                            "AllGather", mybir.AluOpType.bypass,
                            replica_groups=RG,
                            ins=[p_shard[pbuf][:]], outs=[p_full[pbuf][:]])
                    return
                if no_cc:
                    nc.sync.dma_start(
                        p_full[pbuf][NCORES * LO:NCORES * LO + HI, :],
                        p_shard[pbuf][LO:, :])
                else:
                    nc.gpsimd.collective_compute(
                        "AllGather", mybir.AluOpType.bypass, replica_groups=RG,
                        ins=[p_shard[pbuf][LO:, :]],
                        outs=[p_full[pbuf][NCORES * LO:, :]])

            # prologue: projection for layer 0
            for m in range(NBLK):
                emit_p_block(xt_cur, 0, m, 0)
            emit_ag1(0)
            emit_ag2(0)

            # one PSUM bank holds all four 64-col pool-partial accumulation
            # groups plus the final-linear group (5 x 256B sub-bank regions).
            # start=True would zero the whole 2KB zero-region (clobbering the
            # sibling groups), so the bank is memset once and every matmul
            # into it accumulates with start=False.
            pool_bank = psS.tile([P, (KD + 1) * G], F32, tag="poolbank",
                                 name="pool_bank")
            nc.vector.memset(pool_bank[:], 0)
            pool_ps = [pool_bank[:, j * G:(j + 1) * G] for j in range(KD)]
            for l in range(n_layers):
                pbuf = l % 2
                xs_next = xpool.tile([P, NBLK, D], BF, tag="xs")
                last = l == NLAYERS - 1
                if not last:
                    xt_next = xtpool.tile([P, KD, NC_NODES], BF, tag="xt")

                aps_blk = [None] * NBLK

                def emit_root(bb, l=l, aps_blk=aps_blk, xt_cur=xt_cur):
                    """Root x@Ws (+bias) for block bb — needs only resident
                    data, so it can run on PE while the AllGather is in
                    flight."""
                    aps = psA.tile([P, D], F32, tag="aps",
                                   name=f"aps_{l}_{bb}")
                    for k in range(KD):
                        nc.tensor.matmul(
                            aps[:],
                            lhsT=xt_cur[:, k, bb * P:(bb + 1) * P],
                            rhs=ws_sb[:, l, k, :],
                            start=(k == 0), stop=False)
                    if use_bias[l]:
                        nc.tensor.matmul(
                            aps[:], lhsT=ones_sb[:], rhs=bias_sb[:, l, :],
                            start=False, stop=False)
                    aps_blk[bb] = aps

                for j in range(min(lookR, NBLK)):
                    emit_root(j)
                # filler transposes bridge the AllGather wait after the root
                # lookahead runs dry: they keep the PE p-state ramp warm so
                # the first gather-dependent matmuls run at full clock
                # (results are never read; psT banks are overwritten later)
                for w in range(int(os.environ.get("GCN_WARM", "16"))):
                    wps = psT.tile([P, P], BF, tag="tr")
                    nc.tensor.transpose(
                        wps[:], xs_cur[:, 0, :P], ident_sb[:])
                for b in range(NBLK):
                    if b > 0 and b + lookR - 1 < NBLK:
                        emit_root(b + lookR - 1)
                    g = gpool.tile([P, T, D], FP8, tag="g")
                    if no_gather:
                        nc.vector.memset(g[:], 0)
                    else:
                        # split the block gather so segment matmuls on early
                        # tiles overlap later chunks' DMA drain; chunks stay
                        # even so DoubleRow pairs never straddle a chunk
                        col0 = b * (L // 16)
                        # block 0 restarts PE right after the AllGather, so
                        # split its gather for earlier first-tile arrival
                        gc = gchunk if b > 0 else max(2, gchunk // 2)
                        for s0 in range(0, T, gc):
                            s1 = min(s0 + gc, T)
                            nc.gpsimd.dma_gather(
                                g[:, s0:s1, :], p_full[pbuf][:],
                                idxe_sb[:, col0 + s0 * 8:col0 + s1 * 8],
                                (s1 - s0) * P, (s1 - s0) * P, D,
                                single_packet=False)
                    aps = aps_blk[b]
                    # fp8 DoubleRow segment matmuls: each consumes a pair of
                    # 128-edge tiles (K=256) and one 256-wide output half
                    for t in range(0, T, 2):
                        oh_pair = oh_sb[
                            :, (b * T + t) * P:(b * T + t + 2) * P
                        ].rearrange("p (two m) -> p two m", two=2)
                        for h in range(2):
                            nc.tensor.matmul(
                                aps[:, h * HD:(h + 1) * HD],
                                lhsT=oh_pair,
                                rhs=g[:, t:t + 2, h * HD:(h + 1) * HD],
                                start=False,
                                stop=(t + 2 >= T),
                                perf_mode=DR,
                                skip_group_check=True)
                    if last:
                        nc.vector.tensor_tensor(
                            xs_next[:, b, :], aps[:], xs_cur[:, b, :], op=ADD)
                        # pooling partials for this block, interleaved so they
                        # hide under later blocks' gathers
                        for j in range(KD):
                            nc.tensor.matmul(
                                pool_ps[j],
                                lhsT=xs_next[:, b, j * P:(j + 1) * P],
                                rhs=goh_sb[:, b * G:(b + 1) * G],
                                start=False, stop=(b == NBLK - 1),
                                skip_group_check=True)
                    else:
                        t1 = spool.tile([P, D], BF, tag="t1")
                        nc.vector.tensor_tensor(
                            t1[:], aps[:], xs_cur[:, b, :], op=ADD)
                        nc.scalar.activation(
                            xs_next[:, b, :], t1[:],
                            func=mybir.ActivationFunctionType.Relu)
                        # transpose new block into xt_next (channel-major);
                        # the PSUM->SBUF copies ride the Pool engine, which
                        # is otherwise idle (DVE carries the residual adds)
                        for j in range(KD):
                            trps = psT.tile([P, P], BF, tag="tr")
                            nc.tensor.transpose(
                                trps[:], xs_next[:, b, j * P:(j + 1) * P],
                                ident_sb[:])
                            nc.vector.tensor_copy(
                                xt_next[:, j, b * P:(b + 1) * P], trps[:])
                        # pipelined projection for layer l+1, block b
                        emit_p_block(xt_next, l + 1, b, 1 - pbuf)
                        if b == AG1_BLKS - 1:
                            emit_ag1(1 - pbuf)
                if not last:
                    emit_ag2(1 - pbuf)
                    xt_cur = xt_next
                xs_cur = xs_next

            # ---- pooling partials were accumulated inside the last layer's
            # block loop (one PSUM bank per 128-channel chunk). The final
            # linear is applied per-core BEFORE the AllReduce (everything is
            # linear, inv is a per-graph diagonal), shrinking the collective
            # from [P, KD*G] f32 (128KB) to [OUT, G] f32 (32KB).
            poolbf = spool.tile([P, KD * G], BF, tag="poolbf")
            for j in range(KD):
                nc.vector.tensor_tensor(
                    poolbf[:, j * G:(j + 1) * G], pool_ps[j],
                    invt_sb[:, j * G:(j + 1) * G], op=MUL)
            fin_ps = pool_bank[:, KD * G:(KD + 1) * G]
            for k in range(KD):
                nc.tensor.matmul(
                    fin_ps, lhsT=wlin_sb[:, k, :],
                    rhs=poolbf[:, k * G:(k + 1) * G],
                    start=False, stop=(k == KD - 1),
                    skip_group_check=True)
            fin_sb = spool.tile([OUT, G], F32, tag="fin_sb")
            nc.vector.tensor_copy(fin_sb[:], fin_ps)
            nc.sync.dma_start(fin_in[:], fin_sb[:])
            if no_cc:
                nc.sync.dma_start(fin_out[:], fin_in[:])
            else:
                nc.gpsimd.collective_compute(
                    "AllReduce", ADD, replica_groups=RG,
                    ins=[fin_in[:]], outs=[fin_out[:]])
            red_sb = spool.tile([OUT, G], F32, tag="red_sb")
            nc.sync.dma_start(red_sb[:], fin_out[:])
            out_sb = spool.tile([OUT, G], F32, tag="out_sb")
            nc.vector.tensor_tensor(
                out_sb[:], red_sb[:], blin_sb[:, :1].to_broadcast([OUT, G]),
                op=ADD)
            nc.sync.dma_start(out_d[:], out_sb[:])

    nc.compile()
    return nc


def kernel(**inputs):
    import os
    from concourse.bass_utils import run_bass_kernel_spmd

    per_core, shared, T = _prep(inputs)
    use_bias = [bool(np.asarray(inputs[f"b{l+1}"]).astype(np.float32).any())
                for l in range(NLAYERS)]
    nc = _build(T, use_bias)
    in_maps = [{**pc, **shared} for pc in per_core]
    trace = bool(int(os.environ.get("GCN_TRACE", "0")))
    res = run_bass_kernel_spmd(nc, in_maps, core_ids=list(range(NCORES)),
                               trace=trace)
    if trace:
        print(f"HW exec time: {res.exec_time_ns} ns")
        if res.instructions_and_trace is not None:
            print("trace:", res.instructions_and_trace[1])
    out_t = res.results[0]["out_t"]
    return np.ascontiguousarray(out_t.T).astype(np.float32)


# revision 36
# speedup vs baseline: 1.5870x; 1.0008x over previous
"""Distributed GCN (5x GraphConv(add) + residual/ReLU + mean-pool + linear)
for 8 Trainium2 NeuronCores.

Sharding: nodes are permuted so every 128-node block has a near-equal number
of incident (destination) edges, then partitioned contiguously across cores
(1280 nodes/core). Each core owns the edges whose destination lands in its
shard. Aggregation is computed as A@(x@Wr): project first (p = x@Wr),
AllGather p, gather p[src[e]] rows with SWDGE dma_gather, then reduce edge
tiles onto destination nodes with one-hot segment matmuls on the tensor
engine.

The aggregation path runs in fp8 e4m3: p is stored/AllGathered/gathered as
fp8 (|p| < 240 for this model scale) and the one-hot segment matmuls use
DoubleRow perf mode (K=256 contraction, 0.5 PE cycles/row = 2x bf16
throughput). Edge balancing makes T (128-edge tiles per block) uniform and
even, which DoubleRow pairing requires. x@Ws + bias accumulate into the same
PSUM bank; residual+ReLU on DVE/ACT. Weights and node features stay bf16.

The AllGather is split 1024+256 rows per core: the big piece is issued as
soon as blocks 0-7 of the next layer's projection are done, overlapping the
tail blocks' aggregation; only the small piece sits on the critical path.
p_full row layout is therefore [8 cores x rows 0:1024 | 8 cores x rows
1024:1280] and the gather indices are computed against that layout. At each
layer boundary, a lookahead of root (x@Ws) matmuls — which need no gathered
data — keeps the PE fed while the AllGather completes.

SWDGE gather dispatch costs ~1us fixed per instruction, so each 128-node
block's 2048 edge rows are fetched in one dma_gather (two for block 0, which
sits right after the AllGather on the critical path). Deferred constant
loads ride the ACT engine's DMA queue so the SP queue stays clear for the
prologue's projection writes.

Mean-pool partials via matmul against a node->graph one-hot; the final
linear is applied per-core before a 32KB AllReduce (valid because mean-pool
and the linear commute with the cross-core sum). All five pool/final PSUM
accumulation groups share one memset bank with start=False accumulation —
start=True would zero the whole 2KB zero-region and clobber the siblings.
"""

import numpy as np
import ml_dtypes

BF16 = ml_dtypes.bfloat16
E4M3 = ml_dtypes.float8_e4m3

N, E, D, OUT, G = 10000, 160000, 512, 128, 64
NCORES, P = 8, 128
NBLK = 10                     # 128-node blocks per core
NC_NODES = NBLK * P           # 1280
NPAD = NCORES * NC_NODES      # 10240
NLAYERS = 5
KD = D // P                   # 4 chunks of in-channels
HD = D // 2                   # DoubleRow output half-width (256)
AG1_BLKS = 8                  # blocks covered by the early AllGather piece


def _wrap_idx(a):
    """[L] ints -> [128, L//16] int16 SWDGE index layout (16-partition wrap,
    replicated for the 8 Q7 cores)."""
    L = len(a)
    w = a.astype(np.int16).reshape(L // 16, 16).T
    return np.ascontiguousarray(np.tile(w, (8, 1)))


def _balance_nodes(dst):
    """Greedy multiway partition: assign nodes to 80 bins of exactly 128
    slots, minimizing the max per-bin incident-edge count. Returns
    new2old[NPAD] (old node id or -1 for padding)."""
    import heapq

    nbins = NCORES * NBLK
    deg = np.bincount(dst, minlength=N)
    order = np.argsort(-deg, kind="stable")
    cnt = np.zeros(nbins, np.int64)
    edges = np.zeros(nbins, np.int64)
    heap = [(0, b) for b in range(nbins)]
    heapq.heapify(heap)
    node_lists = [[] for _ in range(nbins)]
    for v in order:
        while True:
            _, b = heapq.heappop(heap)
            if cnt[b] < P:
                break
        node_lists[b].append(v)
        cnt[b] += 1
        edges[b] += deg[v]
        if cnt[b] < P:
            heapq.heappush(heap, (edges[b], b))
    new2old = np.full(NPAD, -1, np.int64)
    for b in range(nbins):
        lst = node_lists[b]
        new2old[b * P:b * P + len(lst)] = lst
    return new2old, int(edges.max())


def _ag_split():
    import os
    return bool(int(os.environ.get("GCN_AG_SPLIT", "1")))


def _row_of_new(j):
    """p_full row index for permuted node position j (split-AllGather
    layout: [8 cores x rows 0:1024 | 8 cores x rows 1024:1280])."""
    c, r = j // NC_NODES, j % NC_NODES
    if not _ag_split():
        return c * NC_NODES + r
    lo = AG1_BLKS * P
    return np.where(r < lo, c * lo + r,
                    NCORES * lo + c * (NC_NODES - lo) + (r - lo))


def _prep(inputs):
    x = np.asarray(inputs["x"], np.float32)
    ei = np.asarray(inputs["edge_index"]).astype(np.int64)
    batch = np.asarray(inputs["batch"]).astype(np.int64)
    src, dst = ei[0], ei[1]

    new2old, max_edges = _balance_nodes(dst)
    old2new = np.full(N, -1, np.int64)
    valid = new2old >= 0
    old2new[new2old[valid]] = np.nonzero(valid)[0]

    T = max(2, int(np.ceil(max_edges / P)))
    T += T % 2                     # DoubleRow consumes tile pairs
    L = T * P

    dst_new = old2new[dst]
    src_row = _row_of_new(old2new[src])
    order = np.argsort(dst_new, kind="stable")
    ds_, sr_ = dst_new[order], src_row[order]
    starts = np.searchsorted(ds_, np.arange(0, NPAD + 1, P))

    xp = np.zeros((NPAD, D), np.float32)
    xp[valid] = x[new2old[valid]]
    batch_new = np.full(NPAD, -1, np.int64)
    batch_new[valid] = batch[new2old[valid]]

    counts_g = np.bincount(batch, minlength=G)[:G]
    inv = (1.0 / np.maximum(counts_g, 1.0)).astype(np.float32)

    per_core = []
    for c in range(NCORES):
        idx_blocks = []
        oh_flat = np.zeros((P, NBLK * L), E4M3)
        goh = np.zeros((P, NBLK * G), BF16)
        for b in range(NBLK):
            gb = c * NBLK + b
            lo = gb * P
            s0, s1 = int(starts[gb]), int(starts[gb + 1])
            n = s1 - s0
            assert n <= L
            srcs = np.zeros(L, np.int64)
            srcs[:n] = sr_[s0:s1]
            dloc = ds_[s0:s1] - lo
            oh = np.zeros((L, P), E4M3)
            oh[np.arange(n), dloc] = 1
            idx_blocks.append(_wrap_idx(srcs))
            oh_flat[:, b * L:(b + 1) * L] = (
                oh.reshape(T, P, P).transpose(1, 0, 2).reshape(P, L))
            nodes = lo + np.arange(P)
            bt = batch_new[nodes]
            ok = bt >= 0
            goh[ok, b * G + bt[ok]] = 1

        shard = xp[c * NC_NODES:(c + 1) * NC_NODES].astype(BF16)
        xt0 = np.ascontiguousarray(
            shard.T.reshape(KD, P, NC_NODES).transpose(1, 0, 2))
        per_core.append(dict(
            x_shard=np.ascontiguousarray(shard),
            xt0=xt0,
            ohot=oh_flat,
            idxe=np.ascontiguousarray(np.concatenate(idx_blocks, axis=1)),
            goh=goh,
        ))

    wr = np.zeros((P, NLAYERS, KD, D), BF16)
    ws = np.zeros((P, NLAYERS, KD, D), BF16)
    bias = np.zeros((P, NLAYERS, D), BF16)
    for l in range(NLAYERS):
        wr[:, l] = np.asarray(inputs[f"Wr{l+1}"], np.float32).reshape(
            KD, P, D).transpose(1, 0, 2).astype(BF16)
        ws[:, l] = np.asarray(inputs[f"Ws{l+1}"], np.float32).reshape(
            KD, P, D).transpose(1, 0, 2).astype(BF16)
        bias[0, l] = np.asarray(inputs[f"b{l+1}"], np.float32).astype(BF16)
    ones_e0 = np.zeros((P, P), BF16)
    ones_e0[0, :] = 1
    wlin = np.ascontiguousarray(
        np.asarray(inputs["Wlin"], np.float32).reshape(KD, P, OUT)
        .transpose(1, 0, 2).astype(BF16))
    blin = np.tile(np.asarray(inputs["blin"], np.float32).reshape(OUT, 1),
                   (1, 1)).astype(np.float32)
    shared = dict(
        wr=wr, ws=ws, bias=bias, ones=ones_e0, wlin=wlin, blin=blin,
        invt=np.ascontiguousarray(np.tile(inv, (P, KD)).astype(np.float32)),
        ident=np.eye(P, dtype=BF16),
    )
    return per_core, shared, T


def _unwrap(w, L):
    """inverse of _wrap_idx: [128, L//16] -> [L]"""
    return np.ascontiguousarray(w[:16].T).reshape(-1)[:L].astype(np.int64)


def emulate(inputs):
    """Numpy emulation of the exact device dataflow (bf16/fp8 casts
    included). Validates all host-side index/one-hot bookkeeping."""
    per_core, shared, T = _prep(inputs)
    L = T * P
    f32 = np.float32

    xs = [pc["x_shard"].astype(f32) for pc in per_core]       # [1280, 512]
    for l in range(NLAYERS):
        ws_l = np.concatenate([shared["ws"][:, l, k, :] for k in range(KD)],
                              axis=0).astype(f32)
        wr_l = np.concatenate([shared["wr"][:, l, k, :] for k in range(KD)],
                              axis=0).astype(f32)
        b_l = shared["bias"][0, l].astype(f32)
        # p = x @ Wr, cast fp8, "AllGather" into the p_full row layout
        p_full = np.zeros((NPAD, D), f32)
        for c in range(NCORES):
            p = (xs[c] @ wr_l).astype(E4M3).astype(f32)
            rows = _row_of_new(c * NC_NODES + np.arange(NC_NODES))
            p_full[rows] = p
        new_xs = []
        for c in range(NCORES):
            nx = np.zeros((NC_NODES, D), f32)
            for b in range(NBLK):
                idx = _unwrap(
                    per_core[c]["idxe"][:, b * (L // 16):(b + 1) * (L // 16)], L)
                gath = p_full[idx].astype(E4M3).astype(f32)   # [L, 512]
                acc = np.zeros((P, D), f32)
                for t in range(T):
                    oh = per_core[c]["ohot"][
                        :, (b * T + t) * P:(b * T + t + 1) * P].astype(f32)
                    acc += oh.T @ gath[t * P:(t + 1) * P]
                blk = xs[c][b * P:(b + 1) * P]
                acc += blk @ ws_l + b_l
                val = (acc.astype(f32) + blk)
                if l < NLAYERS - 1:
                    val = np.maximum(val, 0)
                nx[b * P:(b + 1) * P] = val.astype(BF16).astype(f32)
            new_xs.append(nx)
        xs = new_xs
    # pooling
    pooled_T = np.zeros((D, G), f32)
    for c in range(NCORES):
        goh = per_core[c]["goh"].astype(f32)
        for b in range(NBLK):
            blk = xs[c][b * P:(b + 1) * P].astype(BF16).astype(f32)
            for j in range(KD):
                pooled_T[j * P:(j + 1) * P] += (
                    blk[:, j * P:(j + 1) * P].T @ goh[:, b * G:(b + 1) * G])
    inv = shared["invt"][0, :G].astype(f32)
    pooled_T = (pooled_T * inv[None, :]).astype(BF16).astype(f32)
    wlin = np.concatenate([shared["wlin"][:, k, :] for k in range(KD)],
                          axis=0).astype(f32)                 # [512, 128]
    out_T = wlin.T @ pooled_T + shared["blin"][:, :1]         # [128, 64]
    return np.ascontiguousarray(out_T.T).astype(np.float32)


def _build(T, use_bias=None, enable_asserts=False):
    import os
    n_layers = int(os.environ.get("GCN_LAYERS", NLAYERS))
    no_gather = bool(int(os.environ.get("GCN_NO_GATHER", "0")))
    no_cc = bool(int(os.environ.get("GCN_NO_CC", "0")))
    bP = int(os.environ.get("GCN_BANKS", "1"))
    gbufs = int(os.environ.get("GCN_GBUFS", "5"))
    # gather tiles per DMA: SWDGE dispatch is 994ns fixed + 0.34ns/row, so
    # fewer, larger gathers win; 16 = one gather per 128-node block
    gchunk = int(os.environ.get("GCN_GCHUNK", "16"))
    # blocks of root-matmul lookahead: PE work queued ahead of the first
    # gather-dependent matmul, hiding the AllGather at each layer boundary
    lookR = int(os.environ.get("GCN_LOOKAHEAD", "4"))
    if use_bias is None:
        use_bias = [True] * NLAYERS
    import concourse.bass as bass
    import concourse.mybir as mybir
    import concourse.tile as tile
    from concourse import bacc

    F32 = mybir.dt.float32
    BF = mybir.dt.bfloat16
    FP8 = mybir.dt.float8e4
    I16 = mybir.dt.int16
    ADD = mybir.AluOpType.add
    MUL = mybir.AluOpType.mult
    DR = mybir.MatmulPerfMode.DoubleRow
    L = T * P
    LO = AG1_BLKS * P              # 1024 rows in the early AllGather piece
    HI = NC_NODES - LO             # 256 rows in the late piece
    RG = [list(range(NCORES))]

    nc = bacc.Bacc("TRN2", target_bir_lowering=False, debug=False,
                   enable_asserts=enable_asserts, num_devices=NCORES)

    # per-core inputs
    x_d = nc.dram_tensor("x_shard", [NC_NODES, D], BF, kind="ExternalInput")
    xt0_d = nc.dram_tensor("xt0", [P, KD, NC_NODES], BF, kind="ExternalInput")
    oh_d = nc.dram_tensor("ohot", [P, NBLK * L], FP8, kind="ExternalInput")
    idxe_d = nc.dram_tensor("idxe", [P, NBLK * (L // 16)], I16, kind="ExternalInput")
    goh_d = nc.dram_tensor("goh", [P, NBLK * G], BF, kind="ExternalInput")
    # shared inputs
    wr_d = nc.dram_tensor("wr", [P, NLAYERS, KD, D], BF, kind="ExternalInput")
    ws_d = nc.dram_tensor("ws", [P, NLAYERS, KD, D], BF, kind="ExternalInput")
    bias_d = nc.dram_tensor("bias", [P, NLAYERS, D], BF, kind="ExternalInput")
    ones_d = nc.dram_tensor("ones", [P, P], BF, kind="ExternalInput")
    wlin_d = nc.dram_tensor("wlin", [P, KD, OUT], BF, kind="ExternalInput")
    blin_d = nc.dram_tensor("blin", [OUT, 1], F32, kind="ExternalInput")
    invt_d = nc.dram_tensor("invt", [P, KD * G], F32, kind="ExternalInput")
    ident_d = nc.dram_tensor("ident", [P, P], BF, kind="ExternalInput")
    # internal DRAM (double-buffered by layer parity so the AllGather for
    # layer l+1 never WAR-depends on layer l's gathers)
    p_shard = [nc.dram_tensor(f"p_shard{i}", [NC_NODES, D], FP8) for i in (0, 1)]
    p_full = [nc.dram_tensor(f"p_full{i}", [NPAD, D], FP8, addr_space="Shared")
              for i in (0, 1)]
    fin_in = nc.dram_tensor("fin_in", [OUT, G], F32)
    fin_out = nc.dram_tensor("fin_out", [OUT, G], F32, addr_space="Shared")
    # output
    out_d = nc.dram_tensor("out_t", [OUT, G], F32, kind="ExternalOutput")

    with tile.TileContext(nc) as tc:
        with (
            tc.tile_pool(name="const", bufs=1) as const,
            tc.tile_pool(name="xs", bufs=2) as xpool,
            tc.tile_pool(name="xt", bufs=2) as xtpool,
            tc.tile_pool(name="gath", bufs=gbufs) as gpool,
            tc.tile_pool(name="small", bufs=int(os.environ.get("GCN_SBUFS", "4"))) as spool,
            tc.tile_pool(name="psP", bufs=bP, space="PSUM") as psP,
            tc.tile_pool(name="psA", bufs=max(1, lookR), space="PSUM") as psA,
            tc.tile_pool(name="psS", bufs=1, space="PSUM") as psS,
            tc.tile_pool(name="psT", bufs=2, space="PSUM") as psT,
        ):
            # ---- constants to SBUF. DMA order is critical-path order: the
            # layer-0 projection needs only xt0 + Wr[0], so those go first
            # (PE starts ~6us in instead of waiting ~30us for everything);
            # gather tables (idxe/oh) next; remaining layers' weights follow.
            xt_cur = xtpool.tile([P, KD, NC_NODES], BF, tag="xt")
            # head blocks first so the first projection starts ~4us earlier
            nc.sync.dma_start(xt_cur[:, :, :2 * P], xt0_d[:, :, :2 * P])
            wr_sb = const.tile([P, NLAYERS, KD, D], BF, tag="wr")
            nc.sync.dma_start(wr_sb[:, 0], wr_d[:, 0])
            nc.sync.dma_start(xt_cur[:, :, 2 * P:], xt0_d[:, :, 2 * P:])
            ws_sb = const.tile([P, NLAYERS, KD, D], BF, tag="ws")
            nc.sync.dma_start(ws_sb[:, 0], ws_d[:, 0])
            bias_sb = const.tile([P, NLAYERS, D], BF, tag="bias")
            nc.sync.dma_start(bias_sb[:, 0], bias_d[:, 0])
            ones_sb = const.tile([P, P], BF, tag="ones")
            nc.sync.dma_start(ones_sb[:], ones_d[:])
            xs_cur = xpool.tile([P, NBLK, D], BF, tag="xs")
            nc.sync.dma_start(xs_cur[:], x_d.ap().rearrange("(b p) d -> p b d", p=P))
            idxe_sb = const.tile([P, NBLK * (L // 16)], I16, tag="idxe")
            nc.sync.dma_start(idxe_sb[:], idxe_d[:])
            # deferred constants ride the (otherwise idle) ACT engine's DMA
            # queue so the SP queue stays clear for the prologue's p_sb
            # writes + p_full copy — the critical chain to the first gather
            # per-block pieces so this 2.6MB load never monopolizes the DMA
            # aggregate while the prologue's critical transfers queue behind
            oh_sb = const.tile([P, NBLK * L], FP8, tag="oh")
            for bb in range(NBLK):
                nc.scalar.dma_start(oh_sb[:, bb * L:(bb + 1) * L],
                                    oh_d[:, bb * L:(bb + 1) * L])
            ident_sb = const.tile([P, P], BF, tag="ident")
            nc.scalar.dma_start(ident_sb[:], ident_d[:])
            for ll in range(1, NLAYERS):
                nc.scalar.dma_start(wr_sb[:, ll], wr_d[:, ll])
                nc.scalar.dma_start(ws_sb[:, ll], ws_d[:, ll])
                nc.scalar.dma_start(bias_sb[:, ll], bias_d[:, ll])
            goh_sb = const.tile([P, NBLK * G], BF, tag="goh")
            nc.scalar.dma_start(goh_sb[:], goh_d[:])
            wlin_sb = const.tile([P, KD, OUT], BF, tag="wlin")
            nc.scalar.dma_start(wlin_sb[:], wlin_d[:])
            blin_sb = const.tile([OUT, 1], F32, tag="blin")
            nc.scalar.dma_start(blin_sb[:], blin_d[:])
            invt_sb = const.tile([P, KD * G], F32, tag="invt")
            nc.scalar.dma_start(invt_sb[:], invt_d[:])

            def emit_p_block(xt_src, layer, m, pbuf):
                """p[l=layer] block m = x_l[block m] @ Wr_l, into p_shard[pbuf]."""
                pps = psP.tile([P, D], F32, tag="pps", name=f"pps_{layer}_{m}")
                for k in range(KD):
                    nc.tensor.matmul(
                        pps[:],
                        lhsT=xt_src[:, k, m * P:(m + 1) * P],
                        rhs=wr_sb[:, layer, k, :],
                        start=(k == 0), stop=(k == KD - 1))
                p_sb = spool.tile([P, D], FP8, tag="psb", name=f"psb_{layer}_{m}")
                nc.vector.tensor_copy(p_sb[:], pps[:])
                nc.sync.dma_start(
                    p_shard[pbuf][m * P:(m + 1) * P, :], p_sb[:])

            ag_split = _ag_split()

            def emit_ag1(pbuf):
                if not ag_split:
                    return
                if no_cc:
                    nc.sync.dma_start(
                        p_full[pbuf][:LO, :], p_shard[pbuf][:LO, :])
                else:
                    nc.gpsimd.collective_compute(
                        "AllGather", mybir.AluOpType.bypass, replica_groups=RG,
                        ins=[p_shard[pbuf][:LO, :]],
                        outs=[p_full[pbuf][:NCORES * LO, :]])

            def emit_ag2(pbuf):
                if not ag_split:
                    if no_cc:
                        nc.sync.dma_start(
                            p_full[pbuf][:NC_NODES, :], p_shard[pbuf][:])
                    else:
                        nc.gpsimd.collective_compute(
                            "AllGather", mybir.AluOpType.bypass,
                            replica_groups=RG,
                            ins=[p_shard[pbuf][:]], outs=[p_full[pbuf][:]])
                    return
                if no_cc:
                    nc.sync.dma_start(
                        p_full[pbuf][NCORES * LO:NCORES * LO + HI, :],
                        p_shard[pbuf][LO:, :])
                else:
                    nc.gpsimd.collective_compute(
                        "AllGather", mybir.AluOpType.bypass, replica_groups=RG,
                        ins=[p_shard[pbuf][LO:, :]],
                        outs=[p_full[pbuf][NCORES * LO:, :]])

            # prologue: projection for layer 0
            for m in range(NBLK):
                emit_p_block(xt_cur, 0, m, 0)
            emit_ag1(0)
            emit_ag2(0)

            # one PSUM bank holds all four 64-col pool-partial accumulation
            # groups plus the final-linear group (5 x 256B sub-bank regions).
            # start=True would zero the whole 2KB zero-region (clobbering the
            # sibling groups), so the bank is memset once and every matmul
            # into it accumulates with start=False.
            pool_bank = psS.tile([P, (KD + 1) * G], F32, tag="poolbank",
                                 name="pool_bank")
            nc.vector.memset(pool_bank[:], 0)
            pool_ps = [pool_bank[:, j * G:(j + 1) * G] for j in range(KD)]
            for l in range(n_layers):
                pbuf = l % 2
                xs_next = xpool.tile([P, NBLK, D], BF, tag="xs")
                last = l == NLAYERS - 1
                if not last:
                    xt_next = xtpool.tile([P, KD, NC_NODES], BF, tag="xt")

                aps_blk = [None] * NBLK

                def emit_root(bb, l=l, aps_blk=aps_blk, xt_cur=xt_cur):
                    """Root x@Ws (+bias) for block bb — needs only resident
                    data, so it can run on PE while the AllGather is in
                    flight."""
                    aps = psA.tile([P, D], F32, tag="aps",
                                   name=f"aps_{l}_{bb}")
                    for k in range(KD):
                        nc.tensor.matmul(
                            aps[:],
                            lhsT=xt_cur[:, k, bb * P:(bb + 1) * P],
                            rhs=ws_sb[:, l, k, :],
                            start=(k == 0), stop=False)
                    if use_bias[l]:
                        nc.tensor.matmul(
                            aps[:], lhsT=ones_sb[:], rhs=bias_sb[:, l, :],
                            start=False, stop=False)
                    aps_blk[bb] = aps

                for j in range(min(lookR, NBLK)):
                    emit_root(j)
                # filler transposes bridge the AllGather wait after the root
                # lookahead runs dry: they keep the PE p-state ramp warm so
                # the first gather-dependent matmuls run at full clock
                # (results are never read; psT banks are overwritten later)
                for w in range(int(os.environ.get("GCN_WARM", "16"))):
                    wps = psT.tile([P, P], BF, tag="tr")
                    nc.tensor.transpose(
                        wps[:], xs_cur[:, 0, :P], ident_sb[:])
                for b in range(NBLK):
                    if b > 0 and b + lookR - 1 < NBLK:
                        emit_root(b + lookR - 1)
                    g = gpool.tile([P, T, D], FP8, tag="g")
                    if no_gather:
                        nc.vector.memset(g[:], 0)
                    else:
                        # split the block gather so segment matmuls on early
                        # tiles overlap later chunks' DMA drain; chunks stay
                        # even so DoubleRow pairs never straddle a chunk
                        col0 = b * (L // 16)
                        # block 0 restarts PE right after the AllGather, so
                        # split its gather for earlier first-tile arrival
                        gc = gchunk if b > 0 else max(2, gchunk // 2)
                        for s0 in range(0, T, gc):
                            s1 = min(s0 + gc, T)
                            nc.gpsimd.dma_gather(
                                g[:, s0:s1, :], p_full[pbuf][:],
                                idxe_sb[:, col0 + s0 * 8:col0 + s1 * 8],
                                (s1 - s0) * P, (s1 - s0) * P, D,
                                single_packet=False)
                    aps = aps_blk[b]
                    # fp8 DoubleRow segment matmuls: each consumes a pair of
                    # 128-edge tiles (K=256) and one 256-wide output half
                    for t in range(0, T, 2):
                        oh_pair = oh_sb[
                            :, (b * T + t) * P:(b * T + t + 2) * P
                        ].rearrange("p (two m) -> p two m", two=2)
                        for h in range(2):
                            nc.tensor.matmul(
                                aps[:, h * HD:(h + 1) * HD],
                                lhsT=oh_pair,
                                rhs=g[:, t:t + 2, h * HD:(h + 1) * HD],
                                start=False,
                                stop=(t + 2 >= T),
                                perf_mode=DR,
                                skip_group_check=True)
                    if last:
                        nc.vector.tensor_tensor(
                            xs_next[:, b, :], aps[:], xs_cur[:, b, :], op=ADD)
                        # pooling partials for this block, interleaved so they
                        # hide under later blocks' gathers
                        for j in range(KD):
                            nc.tensor.matmul(
                                pool_ps[j],
                                lhsT=xs_next[:, b, j * P:(j + 1) * P],
                                rhs=goh_sb[:, b * G:(b + 1) * G],
                                start=False, stop=(b == NBLK - 1),
                                skip_group_check=True)
                    else:
                        t1 = spool.tile([P, D], BF, tag="t1")
                        nc.vector.tensor_tensor(
                            t1[:], aps[:], xs_cur[:, b, :], op=ADD)
                        nc.scalar.activation(
                            xs_next[:, b, :], t1[:],
                            func=mybir.ActivationFunctionType.Relu)
                        # transpose new block into xt_next (channel-major);
                        # the PSUM->SBUF copies ride the Pool engine, which
                        # is otherwise idle (DVE carries the residual adds)
                        for j in range(KD):
                            trps = psT.tile([P, P], BF, tag="tr")
                            nc.tensor.transpose(
                                trps[:], xs_next[:, b, j * P:(j + 1) * P],
                                ident_sb[:])
                            nc.vector.tensor_copy(
                                xt_next[:, j, b * P:(b + 1) * P], trps[:])
                        # pipelined projection for layer l+1, block b
                        emit_p_block(xt_next, l + 1, b, 1 - pbuf)
                        if b == AG1_BLKS - 1:
                            emit_ag1(1 - pbuf)
                if not last:
                    emit_ag2(1 - pbuf)
                    xt_cur = xt_next
                xs_cur = xs_next

            # ---- pooling partials were accumulated inside the last layer's
            # block loop (one PSUM bank per 128-channel chunk). The final
            # linear is applied per-core BEFORE the AllReduce (everything is
            # linear, inv is a per-graph diagonal), shrinking the collective
            # from [P, KD*G] f32 (128KB) to [OUT, G] f32 (32KB).
            poolbf = spool.tile([P, KD * G], BF, tag="poolbf")
            for j in range(KD):
                nc.vector.tensor_tensor(
                    poolbf[:, j * G:(j + 1) * G], pool_ps[j],
                    invt_sb[:, j * G:(j + 1) * G], op=MUL)
            fin_ps = pool_bank[:, KD * G:(KD + 1) * G]
            for k in range(KD):
                nc.tensor.matmul(
                    fin_ps, lhsT=wlin_sb[:, k, :],
                    rhs=poolbf[:, k * G:(k + 1) * G],
                    start=False, stop=(k == KD - 1),
                    skip_group_check=True)
            fin_sb = spool.tile([OUT, G], F32, tag="fin_sb")
            nc.vector.tensor_copy(fin_sb[:], fin_ps)
            nc.sync.dma_start(fin_in[:], fin_sb[:])
            if no_cc:
                nc.sync.dma_start(fin_out[:], fin_in[:])
            else:
                nc.gpsimd.collective_compute(
                    "AllReduce", ADD, replica_groups=RG,
                    ins=[fin_in[:]], outs=[fin_out[:]])
            red_sb = spool.tile([OUT, G], F32, tag="red_sb")
            nc.sync.dma_start(red_sb[:], fin_out[:])
            out_sb = spool.tile([OUT, G], F32, tag="out_sb")
            nc.vector.tensor_tensor(
                out_sb[:], red_sb[:], blin_sb[:, :1].to_broadcast([OUT, G]),
                op=ADD)
            nc.sync.dma_start(out_d[:], out_sb[:])

    nc.compile()
    return nc


def kernel(**inputs):
    import os
    from concourse.bass_utils import run_bass_kernel_spmd

    per_core, shared, T = _prep(inputs)
    use_bias = [bool(np.asarray(inputs[f"b{l+1}"]).astype(np.float32).any())
                for l in range(NLAYERS)]
    nc = _build(T, use_bias)
    in_maps = [{**pc, **shared} for pc in per_core]
    trace = bool(int(os.environ.get("GCN_TRACE", "0")))
    res = run_bass_kernel_spmd(nc, in_maps, core_ids=list(range(NCORES)),
                               trace=trace)
    if trace:
        print(f"HW exec time: {res.exec_time_ns} ns")
        if res.instructions_and_trace is not None:
            print("trace:", res.instructions_and_trace[1])
    out_t = res.results[0]["out_t"]
    return np.ascontiguousarray(out_t.T).astype(np.float32)
